# revision 1
# baseline (speedup 1.0000x reference)
"""Trainium2 Bass kernel for DihedralAngleEncoder.

Computes phi/psi/omega backbone dihedral sin/cos features and projects
them 6->64 with a linear layer, for coords [64, 4096, 4, 3].

Math notes (vs. the jax reference):
  - cos(sign*arccos(c)) == c, and sin(sign*arccos(c)) == sign*sqrt(1-c^2),
    so arccos/sin/cos are never evaluated.
  - sign(n1_normalized . v3) == sign(n1 . v3) (norms are positive).
  - The boundary duplications (phi at i==0, psi/omega at i==L-1) are
    realized with padded shifted loads; omega at i==L-1 degenerates to
    exactly sin=0, cos=1 which is patched in as constants.

Sharding: pure data parallel over the batch dim, 8 batch rows per core.
Each core processes 32768 positions laid out as SBUF [128 part, 256 col]
(pos = p*256 + t), pipelined in 2 column chunks. The 6->64 projection
runs on the PE with the feature chunk as the stationary operand
([56,128]: 8 blocks of (6 feats + ones row)) against a block-diagonal
[56, 512] weight built from W and b, so PSUM comes out position-major
and DMAs straight to HBM.
"""

import sys
from contextlib import ExitStack

import numpy as np

if "/opt/trn_rl_repo" not in sys.path:
    sys.path.insert(0, "/opt/trn_rl_repo")

B, L = 64, 4096
NCORES = 8
PB = B // NCORES            # batch rows per core
NPOS = PB * L               # 32768 positions per core
P = 128                     # SBUF partitions
T = NPOS // P               # 256 cols per partition
NCH = 2                     # col chunks (pipeline stages)
TC = T // NCH               # 128 cols per chunk
EPS = 1e-8

_CACHE = {}


def _build_module():
    import concourse.bass as bass
    import concourse.bacc as bacc
    import concourse.tile as tile
    from concourse import mybir

    f32 = mybir.dt.float32
    Alu = mybir.AluOpType
    Act = mybir.ActivationFunctionType

    nc = bacc.Bacc(trn_type="TRN2")
    coords = nc.dram_tensor("coords", [P, T * 12], f32, kind="ExternalInput")
    w8 = nc.dram_tensor("w8", [56, 512], f32, kind="ExternalInput")
    out = nc.dram_tensor("out", [NPOS, 64], f32, kind="ExternalOutput")

    with tile.TileContext(nc) as tc, ExitStack() as ctx:
        singles = ctx.enter_context(tc.tile_pool(name="singles", bufs=1))
        work = ctx.enter_context(tc.tile_pool(name="work", bufs=2))
        psum = ctx.enter_context(tc.tile_pool(name="psum", bufs=2, space="PSUM"))
        outp = ctx.enter_context(tc.tile_pool(name="outp", bufs=2))

        # ---- input: padded coords tile ----
        # cols 0..11 left pad (prev position), 12..3083 main, 3084..3095
        # right pad (next position), 3096..3103 unused.
        X = singles.tile([P, 3104], f32)
        nc.vector.memset(X[0:1, 0:12], 0.0)
        nc.vector.memset(X[96:128, 3084:3104], 0.0)
        nc.sync.dma_start(out=X[1:128, 0:12], in_=coords[0:127, 3060:3072])
        # row-first: left-pad C := C[row,0] on partitions 0,16,..
        nc.sync.dma_start(out=X[0:128:16, 6:9], in_=coords[0:128:16, 6:9])
        nc.sync.dma_start(out=X[0:127, 3084:3096], in_=coords[1:128, 0:12])
        # row-last: right-pad N := N[row,4095] on partitions 15,31,..
        nc.sync.dma_start(
            out=X[15:128:16, 3084:3087], in_=coords[15:128:16, 3060:3063]
        )
        nc.sync.dma_start(out=X[:, 12:1560], in_=coords[:, 0:1548])
        nc.sync.dma_start(out=X[:, 1560:3084], in_=coords[:, 1548:3072])

        W8sb = singles.tile([56, 512], f32)
        nc.sync.dma_start(out=W8sb, in_=w8[:, :])

        CONST = singles.tile([P, 2], f32)
        nc.vector.memset(CONST[:, 0:1], 0.0)
        nc.vector.memset(CONST[:, 1:2], 1.0)

        W3 = 3 * TC
        for ch in range(NCH):
            xb = 12 + 12 * TC * ch

            def Xc(off):
                return X[:, off : off + 12 * TC].rearrange("p (t c) -> p c t", c=12)

            # ---- stage 1: difference vectors a,b,c,d,bn (SoA planes) ----
            V = work.tile([P, 5 * W3], f32, tag="V")
            V4 = V.rearrange("p (v k t) -> p v k t", v=5, k=3)
            Vk = V.rearrange("p (q t) -> p q t", t=TC)
            nc.vector.tensor_sub(Vk[:, 0:3, :], Xc(xb)[:, 0:3, :], Xc(xb - 6)[:, 0:3, :])
            nc.vector.tensor_sub(Vk[:, 3:9, :], Xc(xb + 3)[:, 0:6, :], Xc(xb)[:, 0:6, :])
            nc.vector.tensor_sub(
                Vk[:, 9:12, :], Xc(xb + 12)[:, 0:3, :], Xc(xb + 6)[:, 0:3, :]
            )
            nc.vector.tensor_sub(
                Vk[:, 12:15, :], Xc(xb + 15)[:, 0:3, :], Xc(xb + 12)[:, 0:3, :]
            )

            # ---- stage 2: cross products A=axb, B=bxc, C=cxd, M=dxbn ----
            T1 = work.tile([P, 4 * W3], f32, tag="T1")
            T2 = work.tile([P, 4 * W3], f32, tag="T2")
            T14 = T1.rearrange("p (x k t) -> p x k t", x=4, k=3)
            T24 = T2.rearrange("p (x k t) -> p x k t", x=4, k=3)
            for k in range(3):
                p1, p2 = (k + 1) % 3, (k + 2) % 3
                nc.vector.tensor_mul(
                    T14[:, :, k, :], V4[:, 0:4, p1, :], V4[:, 1:5, p2, :]
                )
                nc.vector.tensor_mul(
                    T24[:, :, k, :], V4[:, 0:4, p2, :], V4[:, 1:5, p1, :]
                )
            XP = T1
            nc.vector.tensor_sub(XP, T1, T2)

            # ---- stage 3: dot products ----
            # PR packs 10 dots * 3 comps:
            # [A.B, B.C, C.M | A.A, B.B, C.C, M.M | A.c, B.d, C.bn]
            PR = work.tile([P, 10 * W3], f32, tag="PR")
            nc.vector.tensor_mul(PR[:, 0 : 3 * W3], XP[:, 0 : 3 * W3], XP[:, W3 : 4 * W3])
            nc.gpsimd.tensor_mul(PR[:, 3 * W3 : 7 * W3], XP, XP)
            nc.vector.tensor_mul(
                PR[:, 7 * W3 : 10 * W3], XP[:, 0 : 3 * W3], V[:, 2 * W3 : 5 * W3]
            )
            PR4 = PR.rearrange("p (d k t) -> p d k t", d=10, k=3)
            DOT = work.tile([P, 10 * TC], f32, tag="DOT")
            DOT2 = DOT.rearrange("p (d t) -> p d t", d=10)
            nc.gpsimd.tensor_add(DOT2, PR4[:, :, 0, :], PR4[:, :, 1, :])
            nc.gpsimd.tensor_add(DOT2, DOT2, PR4[:, :, 2, :])

            # ---- stage 4: angles -> sin/cos features ----
            Q = DOT[:, 0 : 3 * TC]          # numerators
            Pn = DOT[:, 3 * TC : 7 * TC]    # squared norms
            S = DOT[:, 7 * TC : 10 * TC]    # sign dots
            R = work.tile([P, 4 * TC], f32, tag="R")
            nc.scalar.activation(R, Pn, Act.Sqrt)
            nc.vector.tensor_scalar_add(R, R, EPS)
            DEN = work.tile([P, 3 * TC], f32, tag="DEN")
            nc.vector.tensor_mul(DEN, R[:, 0 : 3 * TC], R[:, TC : 4 * TC])
            INV = work.tile([P, 3 * TC], f32, tag="INV")
            SCR = work.tile([P, 3 * TC], f32, tag="SCR")
            nc.vector.reciprocal_approx_accurate(out=INV, in_=DEN, scratch=SCR)
            CRAW = work.tile([P, 3 * TC], f32, tag="SCR2")
            nc.vector.tensor_mul(CRAW, Q, INV)

            # F: 7 feature planes: sin(phi,psi,om), cos(phi,psi,om), ones
            F = work.tile([P, 7 * TC], f32, tag="F")
            nc.vector.tensor_scalar(
                F[:, 3 * TC : 6 * TC], CRAW, -1.0, 1.0, op0=Alu.max, op1=Alu.min
            )
            C2 = work.tile([P, 3 * TC], f32, tag="C2")
            nc.scalar.activation(C2, F[:, 3 * TC : 6 * TC], Act.Square)
            nc.vector.tensor_scalar(C2, C2, -1.0, 1.0, op0=Alu.mult, op1=Alu.add)
            nc.scalar.activation(C2, C2, Act.Sqrt)
            SG = work.tile([P, 3 * TC], f32, tag="SG")
            nc.scalar.activation(SG, S, Act.Sign)
            nc.vector.tensor_mul(F[:, 0 : 3 * TC], SG, C2)
            nc.vector.memset(F[:, 6 * TC : 7 * TC], 1.0)

            if ch == NCH - 1:
                # omega at the last residue of each row: sin=0, cos=1.
                nc.sync.dma_start(
                    out=F[15:128:16, 3 * TC - 1 : 3 * TC], in_=CONST[15:128:16, 0:1]
                )
                nc.sync.dma_start(
                    out=F[15:128:16, 6 * TC - 1 : 6 * TC], in_=CONST[15:128:16, 1:2]
                )

            # ---- stationary operand: T56 [56, 16 supergroups * 128] ----
            # supergroup sg covers positions (8*sg + c)*256 + TC*ch + q;
            # T56 row 7c+j holds feature j of block c (j==6 -> ones).
            T56 = work.tile([56, 16 * 128], f32, tag="T56")
            for j in (6, 3, 4, 5, 0, 1, 2):
                for c in range(8):
                    nc.sync.dma_start(
                        out=T56[7 * c + j : 7 * c + j + 1, :],
                        in_=F[c:128:8, j * TC : (j + 1) * TC],
                    )

            # ---- projection: 16 matmuls, PSUM -> SBUF -> HBM ----
            for g in range(4):  # 4 supergroups per psum tile
                ps = psum.tile([P, 2048], f32, tag="ps")
                ob = outp.tile([P, 2048], f32, tag="ob")
                for i in range(4):
                    sg = 4 * g + i
                    nc.tensor.matmul(
                        ps[:, 512 * i : 512 * (i + 1)],
                        lhsT=T56[:, 128 * sg : 128 * (sg + 1)],
                        rhs=W8sb,
                        start=True,
                        stop=True,
                    )
                    nc.scalar.copy(
                        ob[:, 512 * i : 512 * (i + 1)], ps[:, 512 * i : 512 * (i + 1)]
                    )
                # HBM elem offset of (sg=4g+i, c, part q, chan n):
                #   ((8*sg + c)*256 + TC*ch + q)*64 + n
                base = (8 * 4 * g * 256 + TC * ch) * 64
                dst = bass.AP(
                    tensor=out,
                    offset=base,
                    ap=[[64, 128], [8 * 256 * 64, 4], [256 * 64, 8], [1, 64]],
                )
                src = bass.AP(
                    tensor=ob.tensor,
                    offset=ob.offset,
                    ap=[[ob.ap[0][0], 128], [512, 4], [64, 8], [1, 64]],
                )
                nc.sync.dma_start(out=dst, in_=src)

    nc.compile()
    return nc


def _get_nc():
    if "nc" not in _CACHE:
        _CACHE["nc"] = _build_module()
    return _CACHE["nc"]


def _run(in_maps, trace=False, **kw):
    from concourse import bass_utils

    nc = _get_nc()
    return bass_utils.run_bass_kernel_spmd(
        nc, in_maps, core_ids=list(range(NCORES)), trace=trace, **kw
    )


def _make_in_maps(backbone_coords, W, b):
    coords = np.ascontiguousarray(backbone_coords, dtype=np.float32)
    W = np.asarray(W, dtype=np.float32)
    b = np.asarray(b, dtype=np.float32)
    # block-diagonal weights: row 7c+j = feature j of block c; row 7c+6 = bias
    w8 = np.zeros((56, 512), dtype=np.float32)
    for c in range(8):
        w8[7 * c : 7 * c + 6, 64 * c : 64 * (c + 1)] = W.T  # [6, 64]
        w8[7 * c + 6, 64 * c : 64 * (c + 1)] = b
    in_maps = []
    for i in range(NCORES):
        sl = coords[PB * i : PB * (i + 1)].reshape(P, T * 12)
        in_maps.append({"coords": sl, "w8": w8})
    return in_maps


def kernel(backbone_coords, W, b):
    in_maps = _make_in_maps(backbone_coords, W, b)
    res = _run(in_maps)
    outs = [r["out"].reshape(PB, L, 64) for r in res.results]
    return np.concatenate(outs, axis=0)



# revision 16
# speedup vs baseline: 2.5150x; 2.5150x over previous
"""Trainium2 Bass kernel for DihedralAngleEncoder.

Computes phi/psi/omega backbone dihedral sin/cos features and projects
them 6->64 with a linear layer, for coords [64, 4096, 4, 3].

Math notes (vs. the jax reference):
  - cos(sign*arccos(c)) == c, and sin(sign*arccos(c)) == sign*sqrt(1-c^2),
    so arccos/sin/cos are never evaluated.
  - sign(n1_normalized . v3) == sign(n1 . v3) (norms are positive).
  - Shift reuse: d4(i) = d1(i+1), d5(i) = d2(i+1), c45(i) = c12(i+1),
    |c45|^2(i) = |c12|^2(i+1) -- only 3 cross products and 9 dots are
    computed, the rest are shifted column views.
  - The boundary duplications (phi at i==0, psi/omega at i==L-1) are
    realized with padded shifted loads; omega at i==L-1 degenerates to
    exactly sin=0, cos=1 which is patched in with memsets.

Sharding: pure data parallel over the batch dim, 8 batch rows per core.
Each core processes 32768 positions laid out as SBUF [128 part, 256 col]
(pos = p*256 + t), pipelined in NCH column chunks. Features are written
as fp16 into an AoS tile G[p, 8*t + j] (j: 3 sins, 3 coss, ones, zero),
batch-transposed with a single xbar DMA-transpose per chunk, and
projected on the PE against a 16-block-diagonal [128, 1024] fp16 weight.
PSUM comes out position-major; staged copies form one fully contiguous
[128, 16KB] HBM store per chunk.
"""

import sys
from contextlib import ExitStack

import numpy as np

if "/opt/trn_rl_repo" not in sys.path:
    sys.path.insert(0, "/opt/trn_rl_repo")

B, L = 64, 4096
NCORES = 8
PB = B // NCORES            # batch rows per core
NPOS = PB * L               # 32768 positions per core
P = 128                     # SBUF partitions
T = NPOS // P               # 256 cols (positions) per partition
NCH = 4                     # col chunks (pipeline stages)
TC = T // NCH               # 64 positions per partition per chunk
H = TC + 1                  # halo width for shift reuse

_CACHE = {}


def _build_module():
    import concourse.bass as bass
    import concourse.bacc as bacc
    import concourse.tile as tile
    from concourse import mybir

    f32 = mybir.dt.float32
    f16 = mybir.dt.float16
    Alu = mybir.AluOpType
    Act = mybir.ActivationFunctionType

    nc = bacc.Bacc(trn_type="TRN2")
    XW = 12 + T * 12 + 24   # left pad + main + right pad + slack
    coords = nc.dram_tensor("coords", [P, XW], f32, kind="ExternalInput")
    w16 = nc.dram_tensor("w16", [128, 1024], f16, kind="ExternalInput")
    out = nc.dram_tensor("out", [NPOS, 64], f32, kind="ExternalOutput")

    with tile.TileContext(nc) as tc, ExitStack() as ctx:
        singles = ctx.enter_context(tc.tile_pool(name="singles", bufs=1))
        work = ctx.enter_context(tc.tile_pool(name="work", bufs=2))
        psum = ctx.enter_context(tc.tile_pool(name="psum", bufs=4, space="PSUM"))

        # ---- input: padded coords tile ----
        # cols 0..11 left pad (prev position), 12..3083 main, 3084..3095
        # right pad (next position), 3096..3103 unused.
        # Partition p covers positions (p%8)*4096 + (p//8)*256 + t, i.e.
        # batch row p%8, block p//8 (host pre-permutes coords to match and
        # precomputes the 12-col left/right neighbor pads, including the
        # boundary duplications).
        X = singles.tile([P, XW], f32)
        W16sb = singles.tile([128, 1024], f16)
        nc.sync.dma_start(out=W16sb, in_=w16[:, :])
        # coords in 4 slices so chunk 0 can start early
        SW = XW // NCH
        for c in range(NCH):
            nc.sync.dma_start(
                out=X[:, SW * c : SW * (c + 1)],
                in_=coords[:, SW * c : SW * (c + 1)],
            )

        def XV(off, ncomp, width):
            # [p, comp, t] view: comp stride 1, position stride 12
            return X[:, off : off + 12 * width].rearrange(
                "p (t c) -> p c t", c=12
            )[:, 0:ncomp, :]

        for ch in range(NCH):
            xb = 12 + 12 * TC * ch

            # ---- stage 1: difference vectors [d1,d2,d3,d4,d5] ----
            D = work.tile([P, 5 * 3 * H], f32, tag="D")
            D4 = D.rearrange("p (v k t) -> p v k t", v=5, k=3)
            # d2 = CA-N, d3 = C-CA (halo'd)
            nc.vector.tensor_sub(D4[:, 1:3, :, :], XV(xb + 3, 6, H), XV(xb, 6, H))
            # d1 = N_i - C_{i-1} (halo'd)
            nc.vector.tensor_sub(D4[:, 0:1, :, :], XV(xb, 3, H), XV(xb - 6, 3, H))
            # d4 = d1 shifted, d5 = d2 shifted (width TC)
            nc.vector.tensor_scalar_mul(
                D4[:, 3:5, :, 0:TC], D4[:, 0:2, :, 1 : TC + 1], 1.0
            )

            # ---- stage 2: cross products [c12, c23, c34] ----
            T1 = work.tile([P, 3 * 3 * H], f32, tag="T1")
            T2 = work.tile([P, 3 * 3 * H], f32, tag="T2")
            T14 = T1.rearrange("p (x k t) -> p x k t", x=3, k=3)
            T24 = T2.rearrange("p (x k t) -> p x k t", x=3, k=3)
            for k in range(3):
                p1, p2 = (k + 1) % 3, (k + 2) % 3
                nc.vector.tensor_mul(
                    T14[:, :, k, :], D4[:, 0:3, p1, :], D4[:, 1:4, p2, :]
                )
                nc.vector.tensor_mul(
                    T24[:, :, k, :], D4[:, 0:3, p2, :], D4[:, 1:4, p1, :]
                )
            XP = T1
            XP4 = T14
            nc.vector.tensor_sub(XP, T1, T2)

            # ---- stage 3: 9 dots: [A.B, B.C, C.M | n12, n23, n34 | S1,S2,S3]
            PR = work.tile([P, 9 * 3 * H], f32, tag="PR")
            PR4 = PR.rearrange("p (d k t) -> p d k t", d=9, k=3)
            nc.vector.tensor_mul(PR4[:, 0:2, :, :], XP4[:, 0:2, :, :], XP4[:, 1:3, :, :])
            # c12 shifted by one position: flat view at offset 1, k-stride H
            XPs = XP[:, 1 : 1 + 3 * H].rearrange("p (k t) -> p k t", k=3)
            nc.vector.tensor_mul(PR4[:, 2, :, :], XP4[:, 2, :, :], XPs)
            nc.gpsimd.tensor_mul(PR4[:, 3:6, :, :], XP4[:, 0:3, :, :], XP4[:, 0:3, :, :])
            nc.gpsimd.tensor_mul(PR4[:, 6:9, :, :], XP4[:, 0:3, :, :], D4[:, 2:5, :, :])
            DOT = work.tile([P, 9 * H], f32, tag="DOT")
            DOT3 = DOT.rearrange("p (d t) -> p d t", d=9)
            nc.gpsimd.tensor_add(DOT3, PR4[:, :, 0, :], PR4[:, :, 1, :])
            nc.gpsimd.tensor_add(DOT3, DOT3, PR4[:, :, 2, :])

            # ---- stage 4: angles -> sin/cos features (width TC) ----
            G = work.tile([P, 8 * TC], f16, tag="G")
            G4 = G.rearrange("p (t j) -> p j t", j=8)
            nc.gpsimd.memset(G4[:, 6, :], 1.0)
            nc.gpsimd.memset(G4[:, 7, :], 0.0)

            PP = work.tile([P, 3 * TC], f32, tag="PP")
            PP2 = PP.rearrange("p (d t) -> p d t", d=3)
            nc.vector.tensor_mul(PP2[:, 0:2, :], DOT3[:, 3:5, 0:TC], DOT3[:, 4:6, 0:TC])
            nc.vector.tensor_mul(
                PP2[:, 2:3, :], DOT3[:, 5:6, 0:TC], DOT3[:, 3:4, 1 : TC + 1]
            )
            RI = work.tile([P, 3 * TC], f32, tag="RI")
            nc.vector.reciprocal_approx_fast(out=RI, in_=PP)
            SRt = work.tile([P, 3 * TC], f32, tag="SRt")
            nc.scalar.activation(SRt, RI, Act.Sqrt)  # 1/(|n1||n2|)
            CR = work.tile([P, 3 * TC], f32, tag="CR")
            SRt2 = SRt.rearrange("p (d t) -> p d t", d=3)
            CR2 = CR.rearrange("p (d t) -> p d t", d=3)
            nc.vector.tensor_mul(CR2, DOT3[:, 0:3, 0:TC], SRt2)
            COSf = work.tile([P, 3 * TC], f32, tag="COSf")
            nc.vector.tensor_scalar(COSf, CR, -1.0, 1.0, op0=Alu.max, op1=Alu.min)
            COS2 = COSf.rearrange("p (d t) -> p d t", d=3)
            nc.scalar.copy(G4[:, 3:6, :], COS2)
            SQf = work.tile([P, 3 * TC], f32, tag="SQf")
            nc.scalar.activation(SQf, COSf, Act.Square)
            SMf = work.tile([P, 3 * TC], f32, tag="SMf")
            nc.scalar.activation(SMf, SQf, Act.Sqrt, bias=1.0, scale=-1.0)
            SGf = work.tile([P, 3 * TC], f32, tag="SGf")
            SG2 = SGf.rearrange("p (d t) -> p d t", d=3)
            nc.scalar.activation(SG2, DOT3[:, 6:9, 0:TC], Act.Sign)
            SM2 = SMf.rearrange("p (d t) -> p d t", d=3)
            nc.vector.tensor_mul(G4[:, 0:3, :], SM2, SG2)

            # ---- transpose: GT[8t'+j, s, p] = G[p, 128s + 8t' + j] ----
            NS = 8 * TC // 128  # subtiles per chunk
            GT = work.tile([P, NS * 128], f16, tag="GT")
            GT3 = GT.rearrange("p (s r) -> p s r", s=NS)
            nc.sync.dma_start(out=GT3, in_=G[:, :], transpose=True)

            # ---- projection + staged copies + one HBM store ----
            OUT = work.tile([P, TC * 64], f32, tag="OUT")
            for s in range(NS):
                ps = psum.tile([P, 1024], f32, tag="ps")
                for h in range(2):
                    nc.tensor.matmul(
                        ps[:, 512 * h : 512 * (h + 1)],
                        lhsT=GT3[:, s, :],
                        rhs=W16sb[:, 512 * h : 512 * (h + 1)],
                        start=True,
                        stop=True,
                    )
                dst = OUT[:, 1024 * s : 1024 * (s + 1)]
                if s % 4 == 3:
                    nc.vector.tensor_scalar_mul(dst, ps, 1.0)
                else:
                    nc.scalar.copy(dst, ps)
            # partition p = 8*bl + r -> position base r*4096 + bl*256
            dstap = bass.AP(
                tensor=out,
                offset=TC * 64 * ch,
                ap=[[256 * 64, 16], [4096 * 64, 8], [1, TC * 64]],
            )
            nc.sync.dma_start(out=dstap, in_=OUT[:, :])

    nc.compile()
    return nc


def _get_nc():
    if "nc" not in _CACHE:
        _CACHE["nc"] = _build_module()
    return _CACHE["nc"]


def _run(in_maps, trace=False, **kw):
    from concourse import bass_utils

    nc = _get_nc()
    return bass_utils.run_bass_kernel_spmd(
        nc, in_maps, core_ids=list(range(NCORES)), trace=trace, **kw
    )


def _make_in_maps(backbone_coords, W, b):
    coords = np.ascontiguousarray(backbone_coords, dtype=np.float32)
    W = np.asarray(W, dtype=np.float32)
    b = np.asarray(b, dtype=np.float32)
    # 16-block-diagonal weights: row 8t+j = feature j of block t,
    # row 8t+6 = bias (ones feature), row 8t+7 = zero.
    w16 = np.zeros((128, 1024), dtype=np.float16)
    for t in range(16):
        w16[8 * t : 8 * t + 6, 64 * t : 64 * (t + 1)] = W.T.astype(np.float16)
        w16[8 * t + 6, 64 * t : 64 * (t + 1)] = b.astype(np.float16)
    # Per-core SBUF image: partition p = 8*block + row holds positions
    # row*4096 + block*256 + t, with 12-col neighbor pads on both sides.
    # Boundary duplications (reference's i==0 / i==L-1 branches):
    #   row-first left pad:  only its C slot is ever read -> own C[0].
    #   row-last right pad:  N' = N[L-1] (psi's p4 dup), and
    #     CA' = N + CA - C  =>  d5 = CA' - N' = -(d3), which makes
    #     omega degenerate to exactly cos=1, sin~0 as in the reference.
    nb = 16  # blocks per row
    in_maps = []
    for i in range(NCORES):
        arr = coords[PB * i : PB * (i + 1)].reshape(PB, L, 12)
        blocks = arr.reshape(PB, nb, 256, 12)
        sl = np.empty((nb, PB, 12 + 256 * 12 + 24), dtype=np.float32)
        sl[:, :, -12:] = 0.0
        sl[:, :, 12:-24] = blocks.transpose(1, 0, 2, 3).reshape(nb, PB, 256 * 12)
        # left pad: prev position (or own position 0 for block 0)
        sl[1:, :, 0:12] = blocks[:, :-1, 255].transpose(1, 0, 2)
        sl[0, :, 0:12] = blocks[:, 0, 0]
        # right pad: next position (or the omega-degenerate pad for block 15)
        sl[:-1, :, -24:-12] = blocks[:, 1:, 0].transpose(1, 0, 2)
        last = blocks[:, -1, 255]  # [PB, 12]
        sl[-1, :, -24:-21] = last[:, 0:3]                      # N' = N
        sl[-1, :, -21:-18] = last[:, 0:3] + last[:, 3:6] - last[:, 6:9]  # CA'
        sl[-1, :, -18:-15] = last[:, 6:9]                      # C' (unused)
        sl[-1, :, -15:-12] = 0.0
        in_maps.append({"coords": sl.reshape(P, -1), "w16": w16})
    return in_maps


def kernel(backbone_coords, W, b):
    in_maps = _make_in_maps(backbone_coords, W, b)
    res = _run(in_maps)
    outs = [r["out"].reshape(PB, L, 64) for r in res.results]
    return np.concatenate(outs, axis=0)


# revision 20
# speedup vs baseline: 2.9759x; 1.1833x over previous
"""Trainium2 Bass kernel for DihedralAngleEncoder.

Computes phi/psi/omega backbone dihedral sin/cos features and projects
them 6->64 with a linear layer, for coords [64, 4096, 4, 3].

Math notes (vs. the jax reference):
  - cos(sign*arccos(c)) == c, and sin(sign*arccos(c)) == sign*sqrt(1-c^2),
    so arccos/sin/cos are never evaluated.
  - sign(n1_normalized . v3) == sign(n1 . v3) (norms are positive).
  - Shift reuse: d4(i) = d1(i+1), d5(i) = d2(i+1), c45(i) = c12(i+1),
    |c45|^2(i) = |c12|^2(i+1) -- only 3 cross products and 9 dots are
    computed, the rest are shifted column views.
  - The boundary duplications (phi at i==0, psi/omega at i==L-1) are
    realized with padded shifted loads; omega at i==L-1 degenerates to
    exactly sin=0, cos=1 which is patched in with memsets.

Sharding: pure data parallel over the batch dim, 8 batch rows per core.
Each core processes 32768 positions laid out as SBUF [128 part, 256 col]
(pos = p*256 + t), pipelined in NCH column chunks. Features are written
as fp16 into an AoS tile G[p, 8*t + j] (j: 3 sins, 3 coss, ones, zero),
batch-transposed with a single xbar DMA-transpose per chunk, and
projected on the PE against a 16-block-diagonal [128, 1024] fp16 weight.
PSUM comes out position-major; staged copies form one fully contiguous
[128, 16KB] HBM store per chunk.
"""

import sys
from contextlib import ExitStack

import numpy as np

if "/opt/trn_rl_repo" not in sys.path:
    sys.path.insert(0, "/opt/trn_rl_repo")

B, L = 64, 4096
NCORES = 8
PB = B // NCORES            # batch rows per core
NPOS = PB * L               # 32768 positions per core
P = 128                     # SBUF partitions
T = NPOS // P               # 256 cols (positions) per partition
NCH = 4                     # col chunks (pipeline stages)
TC = T // NCH               # 64 positions per partition per chunk
H = TC + 1                  # halo width for shift reuse

_CACHE = {}


def _build_module():
    import concourse.bass as bass
    import concourse.bacc as bacc
    import concourse.tile as tile
    from concourse import mybir

    f32 = mybir.dt.float32
    f16 = mybir.dt.float16
    Alu = mybir.AluOpType
    Act = mybir.ActivationFunctionType

    nc = bacc.Bacc(trn_type="TRN2")
    XW = 12 + T * 12 + 24   # left pad + main + right pad + slack
    coords = nc.dram_tensor("coords", [P, XW], f32, kind="ExternalInput")
    w16 = nc.dram_tensor("w16", [128, 1024], f16, kind="ExternalInput")
    out = nc.dram_tensor("out", [NPOS, 64], f32, kind="ExternalOutput")

    with tile.TileContext(nc) as tc, ExitStack() as ctx:
        singles = ctx.enter_context(tc.tile_pool(name="singles", bufs=1))
        work = ctx.enter_context(tc.tile_pool(name="work", bufs=3))
        psum = ctx.enter_context(tc.tile_pool(name="psum", bufs=4, space="PSUM"))

        # ---- input: padded coords tile ----
        # cols 0..11 left pad (prev position), 12..3083 main, 3084..3095
        # right pad (next position), 3096..3103 unused.
        # Partition p covers positions (p%8)*4096 + (p//8)*256 + t, i.e.
        # batch row p%8, block p//8 (host pre-permutes coords to match and
        # precomputes the 12-col left/right neighbor pads, including the
        # boundary duplications).
        X = singles.tile([P, XW], f32)
        W16sb = singles.tile([128, 1024], f16)
        # coords in chunk-aligned slices so chunk c waits only on slices <= c
        bnds = [0, 800, 1568, 2336, XW]
        for c in range(NCH):
            nc.sync.dma_start(
                out=X[:, bnds[c] : bnds[c + 1]],
                in_=coords[:, bnds[c] : bnds[c + 1]],
            )
            if c == 1:
                nc.sync.dma_start(out=W16sb, in_=w16[:, :])

        def XV(off, ncomp, width):
            # [p, comp, t] view: comp stride 1, position stride 12
            return X[:, off : off + 12 * width].rearrange(
                "p (t c) -> p c t", c=12
            )[:, 0:ncomp, :]

        for ch in range(NCH):
            xb = 12 + 12 * TC * ch

            # ---- stage 1: difference vectors [d1,d2,d3,d4,d5] ----
            D = work.tile([P, 5 * 3 * H], f16, tag="D")
            D4 = D.rearrange("p (v k t) -> p v k t", v=5, k=3)
            # d2 = CA-N, d3 = C-CA (halo'd)
            nc.vector.tensor_sub(D4[:, 1:3, :, :], XV(xb + 3, 6, H), XV(xb, 6, H))
            # d1 = N_i - C_{i-1} (halo'd)
            nc.vector.tensor_sub(D4[:, 0:1, :, :], XV(xb, 3, H), XV(xb - 6, 3, H))
            # d4 = d1 shifted, d5 = d2 shifted (width TC)
            nc.vector.tensor_scalar_mul(
                D4[:, 3:5, :, 0:TC], D4[:, 0:2, :, 1 : TC + 1], 1.0
            )

            # ---- stage 2: cross products [c12, c23, c34] ----
            T1 = work.tile([P, 3 * 3 * H], f16, tag="T1")
            T2 = work.tile([P, 3 * 3 * H], f16, tag="T2")
            T14 = T1.rearrange("p (x k t) -> p x k t", x=3, k=3)
            T24 = T2.rearrange("p (x k t) -> p x k t", x=3, k=3)
            for k in range(3):
                p1, p2 = (k + 1) % 3, (k + 2) % 3
                nc.vector.tensor_mul(
                    T14[:, :, k, :], D4[:, 0:3, p1, :], D4[:, 1:4, p2, :]
                )
                nc.vector.tensor_mul(
                    T24[:, :, k, :], D4[:, 0:3, p2, :], D4[:, 1:4, p1, :]
                )
            XP = T1
            XP4 = T14
            nc.vector.tensor_sub(XP, T1, T2)

            # ---- stage 3: 9 dots: [A.B, B.C, C.M | n12, n23, n34 | S1,S2,S3]
            PR = work.tile([P, 9 * 3 * H], f16, tag="PR")
            PR4 = PR.rearrange("p (d k t) -> p d k t", d=9, k=3)
            nc.vector.tensor_mul(PR4[:, 0:2, :, :], XP4[:, 0:2, :, :], XP4[:, 1:3, :, :])
            # c12 shifted by one position: flat view at offset 1, k-stride H
            XPs = XP[:, 1 : 1 + 3 * H].rearrange("p (k t) -> p k t", k=3)
            nc.vector.tensor_mul(PR4[:, 2, :, :], XP4[:, 2, :, :], XPs)
            nc.gpsimd.tensor_mul(PR4[:, 3:6, :, :], XP4[:, 0:3, :, :], XP4[:, 0:3, :, :])
            nc.gpsimd.tensor_mul(PR4[:, 6:9, :, :], XP4[:, 0:3, :, :], D4[:, 2:5, :, :])
            DOT = work.tile([P, 9 * H], f16, tag="DOT")
            DOT3 = DOT.rearrange("p (d t) -> p d t", d=9)
            nc.vector.tensor_add(DOT3, PR4[:, :, 0, :], PR4[:, :, 1, :])
            nc.vector.tensor_add(DOT3, DOT3, PR4[:, :, 2, :])

            # ---- stage 4: angles -> sin/cos features (width TC) ----
            # cos = Q/sqrt(P1*P2); |sin| = sqrt(P1*P2 - Q^2)/sqrt(P1*P2)
            # (so both sqrts pack into ONE Act instruction); the sign of sin
            # is OR'd in from the S dots' sign bits on DVE.
            G = work.tile([P, 8 * TC], f16, tag="G")
            G4 = G.rearrange("p (t j) -> p j t", j=8)
            nc.gpsimd.memset(G4[:, 6, :], 1.0)
            nc.gpsimd.memset(G4[:, 7, :], 0.0)

            SGf = work.tile([P, 3 * TC], f32, tag="SGf")
            nc.scalar.activation(
                SGf.rearrange("p (d t) -> p d t", d=3),
                DOT3[:, 6:9, 0:TC],
                Act.Sign,
            )
            PPr = work.tile([P, 3 * TC], f32, tag="PPr")
            PP2 = PPr.rearrange("p (d t) -> p d t", d=3)
            nc.vector.tensor_mul(PP2[:, 0:2, :], DOT3[:, 3:5, 0:TC], DOT3[:, 4:6, 0:TC])
            nc.vector.tensor_mul(
                PP2[:, 2:3, :], DOT3[:, 5:6, 0:TC], DOT3[:, 3:4, 1 : TC + 1]
            )
            QQ = work.tile([P, 3 * TC], f32, tag="QQ")
            nc.vector.tensor_mul(
                QQ.rearrange("p (d t) -> p d t", d=3),
                DOT3[:, 0:3, 0:TC],
                DOT3[:, 0:3, 0:TC],
            )
            PK = work.tile([P, 6 * TC], f32, tag="PK")  # [1/(P1P2) | P1P2-Q^2]
            nc.vector.reciprocal_approx_fast(out=PK[:, 0 : 3 * TC], in_=PPr)
            nc.vector.scalar_tensor_tensor(
                PK[:, 3 * TC : 6 * TC], QQ, -1.0, PPr, op0=Alu.mult, op1=Alu.add
            )
            nc.vector.tensor_scalar_max(
                PK[:, 3 * TC : 6 * TC], PK[:, 3 * TC : 6 * TC], 0.0
            )
            SQO = work.tile([P, 6 * TC], f32, tag="SQO")  # [1/sqrt(P) | |sin|*sqrt(P)]
            nc.scalar.activation(SQO, PK, Act.Sqrt)
            SR2 = SQO[:, 0 : 3 * TC].rearrange("p (d t) -> p d t", d=3)
            CR = work.tile([P, 3 * TC], f32, tag="CR")
            nc.vector.tensor_mul(
                CR.rearrange("p (d t) -> p d t", d=3), DOT3[:, 0:3, 0:TC], SR2
            )
            COSf = work.tile([P, 3 * TC], f32, tag="COSf")
            nc.vector.tensor_scalar(COSf, CR, -1.0, 1.0, op0=Alu.max, op1=Alu.min)
            COS2 = COSf.rearrange("p (d t) -> p d t", d=3)
            nc.scalar.copy(G4[:, 3:6, :], COS2)
            SMf = work.tile([P, 3 * TC], f32, tag="SMf")
            nc.vector.tensor_mul(SMf, SQO[:, 3 * TC : 6 * TC], SQO[:, 0 : 3 * TC])
            nc.vector.tensor_mul(
                G4[:, 0:3, :],
                SMf.rearrange("p (d t) -> p d t", d=3),
                SGf.rearrange("p (d t) -> p d t", d=3),
            )

            # ---- transpose: GT[8t'+j, s, p] = G[p, 128s + 8t' + j] ----
            NS = 8 * TC // 128  # subtiles per chunk
            GT = work.tile([P, NS * 128], f16, tag="GT")
            GT3 = GT.rearrange("p (s r) -> p s r", s=NS)
            nc.sync.dma_start(out=GT3, in_=G[:, :], transpose=True)

            # ---- projection + staged copies + one HBM store ----
            OUT = work.tile([P, TC * 64], f32, tag="OUT")
            for s in range(NS):
                ps = psum.tile([P, 1024], f32, tag="ps")
                for h in range(2):
                    nc.tensor.matmul(
                        ps[:, 512 * h : 512 * (h + 1)],
                        lhsT=GT3[:, s, :],
                        rhs=W16sb[:, 512 * h : 512 * (h + 1)],
                        start=True,
                        stop=True,
                    )
                dst = OUT[:, 1024 * s : 1024 * (s + 1)]
                if s % 2 == 1:
                    nc.vector.tensor_scalar_mul(dst, ps, 1.0)
                else:
                    nc.scalar.copy(dst, ps)
                if s % 2 == 1:
                    # partition p = 8*bl + r -> position base r*4096 + bl*256
                    half = (s - 1) // 2
                    dstap = bass.AP(
                        tensor=out,
                        offset=TC * 64 * ch + 2048 * half,
                        ap=[[256 * 64, 16], [4096 * 64, 8], [1, 2048]],
                    )
                    nc.sync.dma_start(
                        out=dstap, in_=OUT[:, 2048 * half : 2048 * (half + 1)]
                    )

    nc.compile()
    return nc


def _get_nc():
    if "nc" not in _CACHE:
        _CACHE["nc"] = _build_module()
    return _CACHE["nc"]


def _run(in_maps, trace=False, **kw):
    from concourse import bass_utils

    nc = _get_nc()
    return bass_utils.run_bass_kernel_spmd(
        nc, in_maps, core_ids=list(range(NCORES)), trace=trace, **kw
    )


def _make_in_maps(backbone_coords, W, b):
    coords = np.ascontiguousarray(backbone_coords, dtype=np.float32)
    W = np.asarray(W, dtype=np.float32)
    b = np.asarray(b, dtype=np.float32)
    # 16-block-diagonal weights: row 8t+j = feature j of block t,
    # row 8t+6 = bias (ones feature), row 8t+7 = zero.
    w16 = np.zeros((128, 1024), dtype=np.float16)
    for t in range(16):
        w16[8 * t : 8 * t + 6, 64 * t : 64 * (t + 1)] = W.T.astype(np.float16)
        w16[8 * t + 6, 64 * t : 64 * (t + 1)] = b.astype(np.float16)
    # Per-core SBUF image: partition p = 8*block + row holds positions
    # row*4096 + block*256 + t, with 12-col neighbor pads on both sides.
    # Boundary duplications (reference's i==0 / i==L-1 branches):
    #   row-first left pad:  only its C slot is ever read -> own C[0].
    #   row-last right pad:  N' = N[L-1] (psi's p4 dup), and
    #     CA' = N + CA - C  =>  d5 = CA' - N' = -(d3), which makes
    #     omega degenerate to exactly cos=1, sin~0 as in the reference.
    nb = 16  # blocks per row
    in_maps = []
    for i in range(NCORES):
        arr = coords[PB * i : PB * (i + 1)].reshape(PB, L, 12)
        blocks = arr.reshape(PB, nb, 256, 12)
        sl = np.empty((nb, PB, 12 + 256 * 12 + 24), dtype=np.float32)
        sl[:, :, -12:] = 0.0
        sl[:, :, 12:-24] = blocks.transpose(1, 0, 2, 3).reshape(nb, PB, 256 * 12)
        # left pad: prev position (or own position 0 for block 0)
        sl[1:, :, 0:12] = blocks[:, :-1, 255].transpose(1, 0, 2)
        sl[0, :, 0:12] = blocks[:, 0, 0]
        # right pad: next position (or the omega-degenerate pad for block 15)
        sl[:-1, :, -24:-12] = blocks[:, 1:, 0].transpose(1, 0, 2)
        last = blocks[:, -1, 255]  # [PB, 12]
        sl[-1, :, -24:-21] = last[:, 0:3]                      # N' = N
        sl[-1, :, -21:-18] = last[:, 0:3] + last[:, 3:6] - last[:, 6:9]  # CA'
        sl[-1, :, -18:-15] = last[:, 6:9]                      # C' (unused)
        sl[-1, :, -15:-12] = 0.0
        in_maps.append({"coords": sl.reshape(P, -1), "w16": w16})
    return in_maps


def kernel(backbone_coords, W, b):
    in_maps = _make_in_maps(backbone_coords, W, b)
    res = _run(in_maps)
    outs = [r["out"].reshape(PB, L, 64) for r in res.results]
    return np.concatenate(outs, axis=0)


# revision 33
# speedup vs baseline: 3.2199x; 1.0820x over previous
"""Trainium2 Bass kernel for DihedralAngleEncoder.

Computes phi/psi/omega backbone dihedral sin/cos features and projects
them 6->64 with a linear layer, for coords [64, 4096, 4, 3].

Math notes (vs. the jax reference):
  - cos(sign*arccos(c)) == c, and sin(sign*arccos(c)) == sign*sqrt(1-c^2),
    so arccos/sin/cos are never evaluated.
  - sign(n1_normalized . v3) == sign(n1 . v3) (norms are positive).
  - Shift reuse: d4(i) = d1(i+1), d5(i) = d2(i+1), c45(i) = c12(i+1),
    |c45|^2(i) = |c12|^2(i+1) -- only 3 cross products and 9 dots are
    computed, the rest are shifted column views.
  - The boundary duplications (phi at i==0, psi/omega at i==L-1) are
    realized with padded shifted loads; omega at i==L-1 degenerates to
    exactly sin=0, cos=1 which is patched in with memsets.

Sharding: pure data parallel over the batch dim, 8 batch rows per core.
Each core processes 32768 positions laid out as SBUF [128 part, 256 col]
(pos = p*256 + t), pipelined in NCH column chunks. Features are written
as fp16 into an AoS tile G[p, 8*t + j] (j: 3 sins, 3 coss, ones, zero),
batch-transposed with a single xbar DMA-transpose per chunk, and
projected on the PE against a 16-block-diagonal [128, 1024] fp16 weight.
PSUM comes out position-major; staged copies form one fully contiguous
[128, 16KB] HBM store per chunk.
"""

import sys
from contextlib import ExitStack

import numpy as np

if "/opt/trn_rl_repo" not in sys.path:
    sys.path.insert(0, "/opt/trn_rl_repo")

B, L = 64, 4096
NCORES = 8
PB = B // NCORES            # batch rows per core
NPOS = PB * L               # 32768 positions per core
P = 128                     # SBUF partitions
T = NPOS // P               # 256 cols (positions) per partition
NCH = 4                     # col chunks (pipeline stages)
TC = T // NCH               # 64 positions per partition per chunk
H = TC + 1                  # halo width for shift reuse

_CACHE = {}


def _build_module():
    import concourse.bass as bass
    import concourse.bacc as bacc
    import concourse.tile as tile
    from concourse import mybir

    f32 = mybir.dt.float32
    f16 = mybir.dt.float16
    Alu = mybir.AluOpType
    Act = mybir.ActivationFunctionType

    nc = bacc.Bacc(trn_type="TRN2")
    XW = 12 + T * 12 + 24   # left pad + main + right pad + slack
    coords = nc.dram_tensor("coords", [P, XW], f16, kind="ExternalInput")
    w16 = nc.dram_tensor("w16", [128, 1024], f16, kind="ExternalInput")
    out = nc.dram_tensor("out", [NPOS, 64], f32, kind="ExternalOutput")

    with tile.TileContext(nc) as tc, ExitStack() as ctx:
        singles = ctx.enter_context(tc.tile_pool(name="singles", bufs=1))
        work = ctx.enter_context(tc.tile_pool(name="work", bufs=4))
        psum = ctx.enter_context(tc.tile_pool(name="psum", bufs=4, space="PSUM"))

        # ---- input: padded coords tile ----
        # cols 0..11 left pad (prev position), 12..3083 main, 3084..3095
        # right pad (next position), 3096..3103 unused.
        # Partition p covers positions (p%8)*4096 + (p//8)*256 + t, i.e.
        # batch row p%8, block p//8 (host pre-permutes coords to match and
        # precomputes the 12-col left/right neighbor pads, including the
        # boundary duplications).
        X = singles.tile([P, XW], f16)
        W16sb = singles.tile([128, 1024], f16)
        # coords in chunk-aligned slices so chunk c waits only on slices <= c
        bnds = [0, 800, 1568, 2336, XW]
        for c in range(NCH):
            nc.sync.dma_start(
                out=X[:, bnds[c] : bnds[c + 1]],
                in_=coords[:, bnds[c] : bnds[c + 1]],
            )
            if c == 1:
                nc.sync.dma_start(out=W16sb, in_=w16[:, :])

        def XV(off, ncomp, width):
            # [p, comp, t] view: comp stride 1, position stride 12
            return X[:, off : off + 12 * width].rearrange(
                "p (t c) -> p c t", c=12
            )[:, 0:ncomp, :]

        for ch in range(NCH):
            xb = 12 + 12 * TC * ch

            # ---- stage 1: difference vectors [d1,d2,d3,d4,d5] ----
            D = work.tile([P, 5 * 3 * H], f16, tag="D")
            D4 = D.rearrange("p (v k t) -> p v k t", v=5, k=3)
            # d2 = CA-N, d3 = C-CA (halo'd)
            nc.vector.tensor_sub(D4[:, 1:3, :, :], XV(xb + 3, 6, H), XV(xb, 6, H))
            # d1 = N_i - C_{i-1} (halo'd)
            nc.vector.tensor_sub(D4[:, 0:1, :, :], XV(xb, 3, H), XV(xb - 6, 3, H))
            # d4 = d1 shifted, d5 = d2 shifted (width TC)
            nc.vector.tensor_scalar_mul(
                D4[:, 3:5, :, 0:TC], D4[:, 0:2, :, 1 : TC + 1], 1.0
            )

            # ---- stage 2: cross products [c12, c23, c34] ----
            T1 = work.tile([P, 3 * 3 * H], f16, tag="T1")
            T2 = work.tile([P, 3 * 3 * H], f16, tag="T2")
            T14 = T1.rearrange("p (x k t) -> p x k t", x=3, k=3)
            T24 = T2.rearrange("p (x k t) -> p x k t", x=3, k=3)
            for k in range(3):
                p1, p2 = (k + 1) % 3, (k + 2) % 3
                nc.vector.tensor_mul(
                    T14[:, :, k, :], D4[:, 0:3, p1, :], D4[:, 1:4, p2, :]
                )
                nc.vector.tensor_mul(
                    T24[:, :, k, :], D4[:, 0:3, p2, :], D4[:, 1:4, p1, :]
                )
            XP = T1
            XP4 = T14
            nc.vector.tensor_sub(XP, T1, T2)

            # ---- stage 3: 9 dots: [A.B, B.C, C.M | n12, n23, n34 | S1,S2,S3]
            PR = work.tile([P, 9 * 3 * H], f16, tag="PR")
            PR4 = PR.rearrange("p (d k t) -> p d k t", d=9, k=3)
            nc.vector.tensor_mul(PR4[:, 0:2, :, :], XP4[:, 0:2, :, :], XP4[:, 1:3, :, :])
            # c12 shifted by one position: flat view at offset 1, k-stride H
            XPs = XP[:, 1 : 1 + 3 * H].rearrange("p (k t) -> p k t", k=3)
            nc.vector.tensor_mul(PR4[:, 2, :, :], XP4[:, 2, :, :], XPs)
            nc.gpsimd.tensor_mul(PR4[:, 3:6, :, :], XP4[:, 0:3, :, :], XP4[:, 0:3, :, :])
            nc.gpsimd.tensor_mul(PR4[:, 6:9, :, :], XP4[:, 0:3, :, :], D4[:, 2:5, :, :])
            DOT = work.tile([P, 9 * H], f16, tag="DOT")
            DOT3 = DOT.rearrange("p (d t) -> p d t", d=9)
            nc.vector.tensor_add(DOT3, PR4[:, :, 0, :], PR4[:, :, 1, :])
            nc.vector.tensor_add(DOT3, DOT3, PR4[:, :, 2, :])

            # ---- stage 4: angles -> sin/cos features (width TC) ----
            # cos = Q/sqrt(P1*P2); |sin| = sqrt(P1*P2 - Q^2)/sqrt(P1*P2)
            # (so both sqrts pack into ONE Act instruction); the sign of sin
            # is OR'd in from the S dots' sign bits on DVE.
            G = work.tile([P, 8 * TC], f16, tag="G")
            G4 = G.rearrange("p (t j) -> p j t", j=8)
            nc.gpsimd.memset(G4[:, 6, :], 1.0)
            nc.gpsimd.memset(G4[:, 7, :], 0.0)

            SGf = work.tile([P, 3 * TC], f32, tag="SGf")
            nc.scalar.activation(
                SGf.rearrange("p (d t) -> p d t", d=3),
                DOT3[:, 6:9, 0:TC],
                Act.Sign,
            )
            PPr = work.tile([P, 3 * TC], f32, tag="PPr")
            PP2 = PPr.rearrange("p (d t) -> p d t", d=3)
            nc.vector.tensor_mul(PP2[:, 0:2, :], DOT3[:, 3:5, 0:TC], DOT3[:, 4:6, 0:TC])
            nc.vector.tensor_mul(
                PP2[:, 2:3, :], DOT3[:, 5:6, 0:TC], DOT3[:, 3:4, 1 : TC + 1]
            )
            QQ = work.tile([P, 3 * TC], f32, tag="QQ")
            nc.vector.tensor_mul(
                QQ.rearrange("p (d t) -> p d t", d=3),
                DOT3[:, 0:3, 0:TC],
                DOT3[:, 0:3, 0:TC],
            )
            PK = work.tile([P, 6 * TC], f32, tag="PK")  # [1/(P1P2) | P1P2-Q^2]
            nc.vector.reciprocal_approx_fast(out=PK[:, 0 : 3 * TC], in_=PPr)
            nc.vector.scalar_tensor_tensor(
                PK[:, 3 * TC : 6 * TC], QQ, -1.0, PPr, op0=Alu.mult, op1=Alu.add
            )
            nc.vector.tensor_scalar_max(
                PK[:, 3 * TC : 6 * TC], PK[:, 3 * TC : 6 * TC], 0.0
            )
            SQO = work.tile([P, 6 * TC], f32, tag="SQO")  # [1/sqrt(P) | |sin|*sqrt(P)]
            nc.scalar.activation(SQO, PK, Act.Sqrt)
            SR2 = SQO[:, 0 : 3 * TC].rearrange("p (d t) -> p d t", d=3)
            CR = work.tile([P, 3 * TC], f32, tag="CR")
            nc.vector.tensor_mul(
                CR.rearrange("p (d t) -> p d t", d=3), DOT3[:, 0:3, 0:TC], SR2
            )
            COSf = work.tile([P, 3 * TC], f32, tag="COSf")
            nc.vector.tensor_scalar(COSf, CR, -1.0, 1.0, op0=Alu.max, op1=Alu.min)
            COS2 = COSf.rearrange("p (d t) -> p d t", d=3)
            nc.vector.tensor_scalar_mul(G4[:, 3:6, :], COS2, 1.0)
            SMf = work.tile([P, 3 * TC], f32, tag="SMf")
            nc.vector.tensor_mul(SMf, SQO[:, 3 * TC : 6 * TC], SQO[:, 0 : 3 * TC])
            nc.vector.tensor_mul(
                G4[:, 0:3, :],
                SMf.rearrange("p (d t) -> p d t", d=3),
                SGf.rearrange("p (d t) -> p d t", d=3),
            )

            # ---- transpose: GT[8t'+j, s, p] = G[p, 128s + 8t' + j] ----
            NS = 8 * TC // 128  # subtiles per chunk
            GT = work.tile([P, NS * 128], f16, tag="GT")
            GT3 = GT.rearrange("p (s r) -> p s r", s=NS)
            nc.sync.dma_start(out=GT3, in_=G[:, :], transpose=True)

            # ---- projection + staged copies + one HBM store ----
            OUT = work.tile([P, TC * 64], f32, tag="OUT")
            for s in range(NS):
                ps = psum.tile([P, 1024], f32, tag="ps")
                for h in range(2):
                    nc.tensor.matmul(
                        ps[:, 512 * h : 512 * (h + 1)],
                        lhsT=GT3[:, s, :],
                        rhs=W16sb[:, 512 * h : 512 * (h + 1)],
                        start=True,
                        stop=True,
                    )
                dst = OUT[:, 1024 * s : 1024 * (s + 1)]
                nc.scalar.copy(dst, ps)
                if s % 2 == 1:
                    # partition p = 8*bl + r -> position base r*4096 + bl*256
                    half = (s - 1) // 2
                    dstap = bass.AP(
                        tensor=out,
                        offset=TC * 64 * ch + 2048 * half,
                        ap=[[256 * 64, 16], [4096 * 64, 8], [1, 2048]],
                    )
                    nc.sync.dma_start(
                        out=dstap, in_=OUT[:, 2048 * half : 2048 * (half + 1)]
                    )

    nc.compile()
    return nc


def _get_nc():
    if "nc" not in _CACHE:
        _CACHE["nc"] = _build_module()
    return _CACHE["nc"]


def _run(in_maps, trace=False, **kw):
    from concourse import bass_utils

    nc = _get_nc()
    return bass_utils.run_bass_kernel_spmd(
        nc, in_maps, core_ids=list(range(NCORES)), trace=trace, **kw
    )


def _make_in_maps(backbone_coords, W, b):
    coords = np.ascontiguousarray(backbone_coords, dtype=np.float32)
    W = np.asarray(W, dtype=np.float32)
    b = np.asarray(b, dtype=np.float32)
    # 16-block-diagonal weights: row 8t+j = feature j of block t,
    # row 8t+6 = bias (ones feature), row 8t+7 = zero.
    w16 = np.zeros((128, 1024), dtype=np.float16)
    for t in range(16):
        w16[8 * t : 8 * t + 6, 64 * t : 64 * (t + 1)] = W.T.astype(np.float16)
        w16[8 * t + 6, 64 * t : 64 * (t + 1)] = b.astype(np.float16)
    # Per-core SBUF image: partition p = 8*block + row holds positions
    # row*4096 + block*256 + t, with 12-col neighbor pads on both sides.
    # Boundary duplications (reference's i==0 / i==L-1 branches):
    #   row-first left pad:  only its C slot is ever read -> own C[0].
    #   row-last right pad:  N' = N[L-1] (psi's p4 dup), and
    #     CA' = N + CA - C  =>  d5 = CA' - N' = -(d3), which makes
    #     omega degenerate to exactly cos=1, sin~0 as in the reference.
    nb = 16  # blocks per row
    in_maps = []
    for i in range(NCORES):
        arr = coords[PB * i : PB * (i + 1)].reshape(PB, L, 12)
        blocks = arr.reshape(PB, nb, 256, 12)
        sl = np.empty((nb, PB, 12 + 256 * 12 + 24), dtype=np.float32)  # cast to f16 below
        sl[:, :, -12:] = 0.0
        sl[:, :, 12:-24] = blocks.transpose(1, 0, 2, 3).reshape(nb, PB, 256 * 12)
        # left pad: prev position (or own position 0 for block 0)
        sl[1:, :, 0:12] = blocks[:, :-1, 255].transpose(1, 0, 2)
        sl[0, :, 0:12] = blocks[:, 0, 0]
        # right pad: next position (or the omega-degenerate pad for block 15)
        sl[:-1, :, -24:-12] = blocks[:, 1:, 0].transpose(1, 0, 2)
        last = blocks[:, -1, 255]  # [PB, 12]
        sl[-1, :, -24:-21] = last[:, 0:3]                      # N' = N
        sl[-1, :, -21:-18] = last[:, 0:3] + last[:, 3:6] - last[:, 6:9]  # CA'
        sl[-1, :, -18:-15] = last[:, 6:9]                      # C' (unused)
        sl[-1, :, -15:-12] = 0.0
        in_maps.append({"coords": sl.reshape(P, -1).astype(np.float16), "w16": w16})
    return in_maps


def kernel(backbone_coords, W, b):
    in_maps = _make_in_maps(backbone_coords, W, b)
    res = _run(in_maps)
    outs = [r["out"].reshape(PB, L, 64) for r in res.results]
    return np.concatenate(outs, axis=0)


# revision 40
# speedup vs baseline: 3.4190x; 1.0618x over previous
"""Trainium2 Bass kernel for DihedralAngleEncoder.

Computes phi/psi/omega backbone dihedral sin/cos features and projects
them 6->64 with a linear layer, for coords [64, 4096, 4, 3].

Math notes (vs. the jax reference):
  - cos(sign*arccos(c)) == c, and sin(sign*arccos(c)) == sign*sqrt(1-c^2),
    so arccos/sin/cos are never evaluated.
  - sign(n1_normalized . v3) == sign(n1 . v3) (norms are positive).
  - Shift reuse: d4(i) = d1(i+1), d5(i) = d2(i+1), c45(i) = c12(i+1),
    |c45|^2(i) = |c12|^2(i+1) -- only 3 cross products and 9 dots are
    computed, the rest are shifted column views.
  - The boundary duplications (phi at i==0, psi/omega at i==L-1) are
    realized with padded shifted loads; omega at i==L-1 degenerates to
    exactly sin=0, cos=1 which is patched in with memsets.

Sharding: pure data parallel over the batch dim, 8 batch rows per core.
Each core processes 32768 positions laid out as SBUF [128 part, 256 col]
(pos = p*256 + t), pipelined in NCH column chunks. Features are written
as fp16 into an AoS tile G[p, 8*t + j] (j: 3 sins, 3 coss, ones, zero),
batch-transposed with a single xbar DMA-transpose per chunk, and
projected on the PE against a 16-block-diagonal [128, 1024] fp16 weight.
PSUM comes out position-major; staged copies form one fully contiguous
[128, 16KB] HBM store per chunk.
"""

import sys
from contextlib import ExitStack

import numpy as np

if "/opt/trn_rl_repo" not in sys.path:
    sys.path.insert(0, "/opt/trn_rl_repo")

B, L = 64, 4096
NCORES = 8
PB = B // NCORES            # batch rows per core
NPOS = PB * L               # 32768 positions per core
P = 128                     # SBUF partitions
T = NPOS // P               # 256 cols (positions) per partition
NCH = 4                     # col chunks (pipeline stages)
TC = T // NCH               # 64 positions per partition per chunk
H = TC + 1                  # halo width for shift reuse

_CACHE = {}


def _build_module():
    import concourse.bass as bass
    import concourse.bacc as bacc
    import concourse.tile as tile
    from concourse import mybir

    f32 = mybir.dt.float32
    f16 = mybir.dt.float16
    Alu = mybir.AluOpType
    Act = mybir.ActivationFunctionType

    nc = bacc.Bacc(trn_type="TRN2")
    XW = 12 + T * 12 + 24   # left pad + main + right pad + slack
    coords = nc.dram_tensor("coords", [P, XW], f16, kind="ExternalInput")
    w16 = nc.dram_tensor("w16", [128, 1024], f16, kind="ExternalInput")
    ident = nc.dram_tensor("ident", [128, 128], f16, kind="ExternalInput")
    out = nc.dram_tensor("out", [NPOS, 64], f32, kind="ExternalOutput")

    with tile.TileContext(nc) as tc, ExitStack() as ctx:
        singles = ctx.enter_context(tc.tile_pool(name="singles", bufs=1))
        work = ctx.enter_context(tc.tile_pool(name="work", bufs=4))
        psum = ctx.enter_context(tc.tile_pool(name="psum", bufs=3, space="PSUM"))
        psumt = ctx.enter_context(tc.tile_pool(name="psumt", bufs=2, space="PSUM"))

        # ---- input: padded coords tile ----
        # cols 0..11 left pad (prev position), 12..3083 main, 3084..3095
        # right pad (next position), 3096..3103 unused.
        # Partition p covers positions (p%8)*4096 + (p//8)*256 + t, i.e.
        # batch row p%8, block p//8 (host pre-permutes coords to match and
        # precomputes the 12-col left/right neighbor pads, including the
        # boundary duplications).
        X = singles.tile([P, XW], f16)
        W16sb = singles.tile([128, 1024], f16)
        IDsb = singles.tile([128, 128], f16)
        nc.sync.dma_start(out=IDsb, in_=ident[:, :])
        # coords in chunk-aligned slices so chunk c waits only on slices <= c
        bnds = [0, 800, 1568, 2336, XW]
        for c in range(NCH):
            nc.sync.dma_start(
                out=X[:, bnds[c] : bnds[c + 1]],
                in_=coords[:, bnds[c] : bnds[c + 1]],
            )
            if c == 1:
                nc.sync.dma_start(out=W16sb, in_=w16[:, :])

        def XV(off, ncomp, width):
            # [p, comp, t] view: comp stride 1, position stride 12
            return X[:, off : off + 12 * width].rearrange(
                "p (t c) -> p c t", c=12
            )[:, 0:ncomp, :]

        for ch in range(NCH):
            xb = 12 + 12 * TC * ch

            # ---- stage 1: difference vectors [d1,d2,d3,d4,d5] ----
            D = work.tile([P, 5 * 3 * H], f16, tag="D")
            D4 = D.rearrange("p (v k t) -> p v k t", v=5, k=3)
            # d2 = CA-N, d3 = C-CA (halo'd)
            nc.vector.tensor_sub(D4[:, 1:3, :, :], XV(xb + 3, 6, H), XV(xb, 6, H))
            # d1 = N_i - C_{i-1} (halo'd)
            nc.vector.tensor_sub(D4[:, 0:1, :, :], XV(xb, 3, H), XV(xb - 6, 3, H))
            # d4 = d1 shifted, d5 = d2 shifted (width TC)
            nc.vector.tensor_scalar_mul(
                D4[:, 3:5, :, 0:TC], D4[:, 0:2, :, 1 : TC + 1], 1.0
            )

            # ---- stage 2: cross products [c12, c23, c34] ----
            T1 = work.tile([P, 3 * 3 * H], f16, tag="T1")
            T2 = work.tile([P, 3 * 3 * H], f16, tag="T2")
            T14 = T1.rearrange("p (x k t) -> p x k t", x=3, k=3)
            T24 = T2.rearrange("p (x k t) -> p x k t", x=3, k=3)
            for k in range(3):
                p1, p2 = (k + 1) % 3, (k + 2) % 3
                nc.vector.tensor_mul(
                    T14[:, :, k, :], D4[:, 0:3, p1, :], D4[:, 1:4, p2, :]
                )
                nc.vector.tensor_mul(
                    T24[:, :, k, :], D4[:, 0:3, p2, :], D4[:, 1:4, p1, :]
                )
            XP = T1
            XP4 = T14
            nc.vector.tensor_sub(XP, T1, T2)

            # ---- stage 3: 9 dots: [A.B, B.C, C.M | n12, n23, n34 | S1,S2,S3]
            PR = work.tile([P, 9 * 3 * H], f16, tag="PR")
            PR4 = PR.rearrange("p (d k t) -> p d k t", d=9, k=3)
            nc.vector.tensor_mul(PR4[:, 0:2, :, :], XP4[:, 0:2, :, :], XP4[:, 1:3, :, :])
            # c12 shifted by one position: flat view at offset 1, k-stride H
            XPs = XP[:, 1 : 1 + 3 * H].rearrange("p (k t) -> p k t", k=3)
            nc.vector.tensor_mul(PR4[:, 2, :, :], XP4[:, 2, :, :], XPs)
            nc.gpsimd.tensor_mul(PR4[:, 3:6, :, :], XP4[:, 0:3, :, :], XP4[:, 0:3, :, :])
            nc.gpsimd.tensor_mul(PR4[:, 6:9, :, :], XP4[:, 0:3, :, :], D4[:, 2:5, :, :])
            DOT = work.tile([P, 9 * H], f16, tag="DOT")
            DOT3 = DOT.rearrange("p (d t) -> p d t", d=9)
            nc.vector.tensor_add(DOT3, PR4[:, :, 0, :], PR4[:, :, 1, :])
            nc.vector.tensor_add(DOT3, DOT3, PR4[:, :, 2, :])

            # ---- stage 4: angles -> sin/cos features (width TC) ----
            # cos = Q/sqrt(P1*P2); |sin| = sqrt(P1*P2 - Q^2)/sqrt(P1*P2)
            # (so both sqrts pack into ONE Act instruction); the sign of sin
            # is OR'd in from the S dots' sign bits on DVE.
            G = work.tile([P, 8 * TC], f16, tag="G")
            G4 = G.rearrange("p (t j) -> p j t", j=8)
            nc.gpsimd.memset(G4[:, 6, :], 1.0)
            nc.gpsimd.memset(G4[:, 7, :], 0.0)

            SGf = work.tile([P, 3 * TC], f32, tag="SGf")
            nc.scalar.activation(
                SGf.rearrange("p (d t) -> p d t", d=3),
                DOT3[:, 6:9, 0:TC],
                Act.Sign,
            )
            PPr = work.tile([P, 3 * TC], f32, tag="PPr")
            PP2 = PPr.rearrange("p (d t) -> p d t", d=3)
            nc.vector.tensor_mul(PP2[:, 0:2, :], DOT3[:, 3:5, 0:TC], DOT3[:, 4:6, 0:TC])
            nc.vector.tensor_mul(
                PP2[:, 2:3, :], DOT3[:, 5:6, 0:TC], DOT3[:, 3:4, 1 : TC + 1]
            )
            QQ = work.tile([P, 3 * TC], f32, tag="QQ")
            nc.vector.tensor_mul(
                QQ.rearrange("p (d t) -> p d t", d=3),
                DOT3[:, 0:3, 0:TC],
                DOT3[:, 0:3, 0:TC],
            )
            PK = work.tile([P, 6 * TC], f32, tag="PK")  # [1/(P1P2) | P1P2-Q^2]
            nc.vector.reciprocal_approx_fast(out=PK[:, 0 : 3 * TC], in_=PPr)
            nc.vector.scalar_tensor_tensor(
                PK[:, 3 * TC : 6 * TC], QQ, -1.0, PPr, op0=Alu.mult, op1=Alu.add
            )
            nc.vector.tensor_scalar_max(
                PK[:, 3 * TC : 6 * TC], PK[:, 3 * TC : 6 * TC], 0.0
            )
            SQO = work.tile([P, 6 * TC], f32, tag="SQO")  # [1/sqrt(P) | |sin|*sqrt(P)]
            nc.scalar.activation(SQO, PK, Act.Sqrt)
            SR2 = SQO[:, 0 : 3 * TC].rearrange("p (d t) -> p d t", d=3)
            CR = work.tile([P, 3 * TC], f32, tag="CR")
            nc.vector.tensor_mul(
                CR.rearrange("p (d t) -> p d t", d=3), DOT3[:, 0:3, 0:TC], SR2
            )
            COSf = work.tile([P, 3 * TC], f32, tag="COSf")
            nc.vector.tensor_scalar(COSf, CR, -1.0, 1.0, op0=Alu.max, op1=Alu.min)
            COS2 = COSf.rearrange("p (d t) -> p d t", d=3)
            nc.vector.tensor_scalar_mul(G4[:, 3:6, :], COS2, 1.0)
            SMf = work.tile([P, 3 * TC], f32, tag="SMf")
            nc.vector.tensor_mul(SMf, SQO[:, 3 * TC : 6 * TC], SQO[:, 0 : 3 * TC])
            nc.vector.tensor_mul(
                G4[:, 0:3, :],
                SMf.rearrange("p (d t) -> p d t", d=3),
                SGf.rearrange("p (d t) -> p d t", d=3),
            )

            # ---- transpose on PE: GT_s[8t'+j, p] = G[p, 128s + 8t'+j] ----
            NS = 8 * TC // 128  # subtiles per chunk
            GT = work.tile([P, NS * 128], f16, tag="GT")
            GT3 = GT.rearrange("p (s r) -> p s r", s=NS)
            OUT = work.tile([P, TC * 64], f32, tag="OUT")
            for s in range(NS):
                pst = psumt.tile([P, 128], f16, tag="pst")
                nc.tensor.transpose(pst, G[:, 128 * s : 128 * (s + 1)], IDsb)
                nc.vector.tensor_scalar_mul(GT3[:, s, :], pst, 1.0)
            for s in range(NS):
                ps = psum.tile([P, 1024], f32, tag="ps")
                for h in range(2):
                    nc.tensor.matmul(
                        ps[:, 512 * h : 512 * (h + 1)],
                        lhsT=GT3[:, s, :],
                        rhs=W16sb[:, 512 * h : 512 * (h + 1)],
                        start=True,
                        stop=True,
                    )
                dst = OUT[:, 1024 * s : 1024 * (s + 1)]
                nc.scalar.copy(dst, ps)
                # partition p = 8*bl + r -> position base r*4096 + bl*256
                dstap = bass.AP(
                    tensor=out,
                    offset=TC * 64 * ch + 1024 * s,
                    ap=[[256 * 64, 16], [4096 * 64, 8], [1, 1024]],
                )
                nc.sync.dma_start(
                    out=dstap, in_=OUT[:, 1024 * s : 1024 * (s + 1)]
                )

    nc.compile()
    return nc


def _get_nc():
    if "nc" not in _CACHE:
        _CACHE["nc"] = _build_module()
    return _CACHE["nc"]


def _run(in_maps, trace=False, **kw):
    from concourse import bass_utils

    nc = _get_nc()
    return bass_utils.run_bass_kernel_spmd(
        nc, in_maps, core_ids=list(range(NCORES)), trace=trace, **kw
    )


def _make_in_maps(backbone_coords, W, b):
    coords = np.ascontiguousarray(backbone_coords, dtype=np.float32)
    W = np.asarray(W, dtype=np.float32)
    b = np.asarray(b, dtype=np.float32)
    # 16-block-diagonal weights: row 8t+j = feature j of block t,
    # row 8t+6 = bias (ones feature), row 8t+7 = zero.
    w16 = np.zeros((128, 1024), dtype=np.float16)
    for t in range(16):
        w16[8 * t : 8 * t + 6, 64 * t : 64 * (t + 1)] = W.T.astype(np.float16)
        w16[8 * t + 6, 64 * t : 64 * (t + 1)] = b.astype(np.float16)
    # Per-core SBUF image: partition p = 8*block + row holds positions
    # row*4096 + block*256 + t, with 12-col neighbor pads on both sides.
    # Boundary duplications (reference's i==0 / i==L-1 branches):
    #   row-first left pad:  only its C slot is ever read -> own C[0].
    #   row-last right pad:  N' = N[L-1] (psi's p4 dup), and
    #     CA' = N + CA - C  =>  d5 = CA' - N' = -(d3), which makes
    #     omega degenerate to exactly cos=1, sin~0 as in the reference.
    nb = 16  # blocks per row
    in_maps = []
    for i in range(NCORES):
        arr = coords[PB * i : PB * (i + 1)].reshape(PB, L, 12)
        blocks = arr.reshape(PB, nb, 256, 12)
        sl = np.empty((nb, PB, 12 + 256 * 12 + 24), dtype=np.float32)  # cast to f16 below
        sl[:, :, -12:] = 0.0
        sl[:, :, 12:-24] = blocks.transpose(1, 0, 2, 3).reshape(nb, PB, 256 * 12)
        # left pad: prev position (or own position 0 for block 0)
        sl[1:, :, 0:12] = blocks[:, :-1, 255].transpose(1, 0, 2)
        sl[0, :, 0:12] = blocks[:, 0, 0]
        # right pad: next position (or the omega-degenerate pad for block 15)
        sl[:-1, :, -24:-12] = blocks[:, 1:, 0].transpose(1, 0, 2)
        last = blocks[:, -1, 255]  # [PB, 12]
        sl[-1, :, -24:-21] = last[:, 0:3]                      # N' = N
        sl[-1, :, -21:-18] = last[:, 0:3] + last[:, 3:6] - last[:, 6:9]  # CA'
        sl[-1, :, -18:-15] = last[:, 6:9]                      # C' (unused)
        sl[-1, :, -15:-12] = 0.0
        in_maps.append({
            "coords": sl.reshape(P, -1).astype(np.float16),
            "w16": w16,
            "ident": np.eye(128, dtype=np.float16),
        })
    return in_maps


def kernel(backbone_coords, W, b):
    in_maps = _make_in_maps(backbone_coords, W, b)
    res = _run(in_maps)
    outs = [r["out"].reshape(PB, L, 64) for r in res.results]
    return np.concatenate(outs, axis=0)


# revision 61
# speedup vs baseline: 4.2304x; 1.2373x over previous
"""Trainium2 Bass kernel for DihedralAngleEncoder.

Computes phi/psi/omega backbone dihedral sin/cos features and projects
them 6->64 with a linear layer, for coords [64, 4096, 4, 3].

Math notes (vs. the jax reference):
  - cos(sign*arccos(c)) == c, and sin(sign*arccos(c)) == sign*sqrt(1-c^2),
    so arccos/sin/cos are never evaluated.
  - sign(n1_normalized . v3) == sign(n1 . v3) (norms are positive).
  - Shift reuse: d4(i) = d1(i+1), d5(i) = d2(i+1), c45(i) = c12(i+1),
    |c45|^2(i) = |c12|^2(i+1) -- only 3 cross products and 9 dots are
    computed, the rest are shifted column views.
  - The boundary duplications (phi at i==0, psi/omega at i==L-1) are
    realized with padded shifted loads; omega at i==L-1 degenerates to
    exactly sin=0, cos=1 which is patched in with memsets.

Sharding: pure data parallel over the batch dim, 8 batch rows per core.
Each core processes 32768 positions laid out as SBUF [128 part, 256 col]
(pos = p*256 + t), pipelined in NCH column chunks. Features are written
as fp16 into an AoS tile G[p, 8*t + j] (j: 3 sins, 3 coss, ones, zero),
batch-transposed with a single xbar DMA-transpose per chunk, and
projected on the PE against a 16-block-diagonal [128, 1024] fp16 weight.
PSUM comes out position-major; staged copies form one fully contiguous
[128, 16KB] HBM store per chunk.
"""

import sys
from contextlib import ExitStack

import numpy as np

if "/opt/trn_rl_repo" not in sys.path:
    sys.path.insert(0, "/opt/trn_rl_repo")

B, L = 64, 4096
NCORES = 8
PB = B // NCORES            # batch rows per core
NPOS = PB * L               # 32768 positions per core
P = 128                     # SBUF partitions
T = NPOS // P               # 256 cols (positions) per partition
NCH = 4                     # col chunks (pipeline stages)
TC = T // NCH               # 64 positions per partition per chunk
H = TC + 1                  # halo width for shift reuse

_CACHE = {}


def _build_module():
    import concourse.bass as bass
    import concourse.bacc as bacc
    import concourse.tile as tile
    from concourse import mybir

    f32 = mybir.dt.float32
    f16 = mybir.dt.float16
    Alu = mybir.AluOpType
    Act = mybir.ActivationFunctionType

    nc = bacc.Bacc(trn_type="TRN2")
    XW = 12 + T * 12 + 24   # left pad + main + right pad + slack
    coords = nc.dram_tensor("coords", [P, XW], f16, kind="ExternalInput")
    w16 = nc.dram_tensor("w16", [128, 1024], f16, kind="ExternalInput")
    ident = nc.dram_tensor("ident", [128, 128], f16, kind="ExternalInput")
    out = nc.dram_tensor("out", [NPOS, 64], f32, kind="ExternalOutput")

    with tile.TileContext(nc) as tc, ExitStack() as ctx:
        singles = ctx.enter_context(tc.tile_pool(name="singles", bufs=1))
        work = ctx.enter_context(tc.tile_pool(name="work", bufs=4))
        psum = ctx.enter_context(tc.tile_pool(name="psum", bufs=3, space="PSUM"))
        psumt = ctx.enter_context(tc.tile_pool(name="psumt", bufs=2, space="PSUM"))

        # ---- input: padded coords tile ----
        # cols 0..11 left pad (prev position), 12..3083 main, 3084..3095
        # right pad (next position), 3096..3103 unused.
        # Partition p covers positions (p%8)*4096 + (p//8)*256 + t, i.e.
        # batch row p%8, block p//8 (host pre-permutes coords to match and
        # precomputes the 12-col left/right neighbor pads, including the
        # boundary duplications).
        X = singles.tile([P, XW], f16)
        W16sb = singles.tile([128, 1024], f16)
        IDsb = singles.tile([128, 128], f16)
        nc.sync.dma_start(out=IDsb, in_=ident[:, :])
        # warm both Act function-table sets off the critical path
        TRIG = singles.tile([128, 1], f32)
        nc.scalar.memzero(TRIG)
        nc.scalar.activation(TRIG, TRIG, Act.Sqrt)
        nc.scalar.activation(TRIG, TRIG, Act.Sign)
        # coords in chunk-aligned slices so chunk c waits only on slices <= c
        bnds = [0, 800, 1568, 2336, XW]
        for c in range(NCH):
            nc.sync.dma_start(
                out=X[:, bnds[c] : bnds[c + 1]],
                in_=coords[:, bnds[c] : bnds[c + 1]],
            )
            if c == 1:
                nc.sync.dma_start(out=W16sb, in_=w16[:, :])

        def XV(off, ncomp, width):
            # [p, comp, t] view: comp stride 1, position stride 12
            return X[:, off : off + 12 * width].rearrange(
                "p (t c) -> p c t", c=12
            )[:, 0:ncomp, :]

        for ch in range(NCH):
            xb = 12 + 12 * TC * ch

            # ---- stage 1: difference vectors [d1,d2,d3,d4,d5] ----
            D = work.tile([P, 5 * 3 * H], f16, tag="D")
            D4 = D.rearrange("p (v k t) -> p v k t", v=5, k=3)
            # d2 = CA-N, d3 = C-CA (halo'd)
            nc.vector.tensor_sub(D4[:, 1:3, :, :], XV(xb + 3, 6, H), XV(xb, 6, H))
            # d1 = N_i - C_{i-1} (halo'd)
            nc.vector.tensor_sub(D4[:, 0:1, :, :], XV(xb, 3, H), XV(xb - 6, 3, H))
            # d4 = d1 shifted, d5 = d2 shifted (width TC)
            nc.vector.tensor_scalar_mul(
                D4[:, 3:5, :, 0:TC], D4[:, 0:2, :, 1 : TC + 1], 1.0
            )

            # ---- stage 2: cross products [c12, c23, c34] ----
            T1 = work.tile([P, 3 * 3 * H], f16, tag="T1")
            T2 = work.tile([P, 3 * 3 * H], f16, tag="T2")
            T14 = T1.rearrange("p (x k t) -> p x k t", x=3, k=3)
            T24 = T2.rearrange("p (x k t) -> p x k t", x=3, k=3)
            for k in range(3):
                p1, p2 = (k + 1) % 3, (k + 2) % 3
                nc.vector.tensor_mul(
                    T14[:, :, k, :], D4[:, 0:3, p1, :], D4[:, 1:4, p2, :]
                )
                nc.vector.tensor_mul(
                    T24[:, :, k, :], D4[:, 0:3, p2, :], D4[:, 1:4, p1, :]
                )
            XP = T1
            XP4 = T14
            nc.vector.tensor_sub(XP, T1, T2)

            # ---- stage 3: 9 dots: [A.B, B.C, C.M | n12, n23, n34 | S1,S2,S3]
            PR = work.tile([P, 9 * 3 * H], f16, tag="PR")
            PR4 = PR.rearrange("p (d k t) -> p d k t", d=9, k=3)
            nc.vector.tensor_mul(PR4[:, 0:2, :, :], XP4[:, 0:2, :, :], XP4[:, 1:3, :, :])
            # c12 shifted by one position: flat view at offset 1, k-stride H
            XPs = XP[:, 1 : 1 + 3 * H].rearrange("p (k t) -> p k t", k=3)
            nc.vector.tensor_mul(PR4[:, 2, :, :], XP4[:, 2, :, :], XPs)
            nc.vector.tensor_mul(PR4[:, 3:6, :, :], XP4[:, 0:3, :, :], XP4[:, 0:3, :, :])
            m4eng = nc.vector if ch == 0 else nc.gpsimd
            m4eng.tensor_mul(PR4[:, 6:9, :, :], XP4[:, 0:3, :, :], D4[:, 2:5, :, :])
            DOT = work.tile([P, 9 * H], f16, tag="DOT")
            DOT3 = DOT.rearrange("p (d t) -> p d t", d=9)
            nc.vector.tensor_add(DOT3, PR4[:, :, 0, :], PR4[:, :, 1, :])
            nc.vector.tensor_add(DOT3, DOT3, PR4[:, :, 2, :])

            # ---- stage 4: angles -> sin/cos features (width TC) ----
            # cos = Q/sqrt(P1*P2); |sin| = sqrt(P1*P2 - Q^2)/sqrt(P1*P2)
            # (so both sqrts pack into ONE Act instruction); the sign of sin
            # is OR'd in from the S dots' sign bits on DVE.
            G = work.tile([P, 8 * TC], f16, tag="G")
            G4 = G.rearrange("p (t j) -> p j t", j=8)
            nc.gpsimd.memset(G4[:, 6, :], 1.0)
            nc.gpsimd.memset(G4[:, 7, :], 0.0)

            SGf = work.tile([P, 3 * TC], f32, tag="SGf")
            nc.scalar.activation(
                SGf.rearrange("p (d t) -> p d t", d=3),
                DOT3[:, 6:9, 0:TC],
                Act.Sign,
            )
            PPr = work.tile([P, 3 * TC], f32, tag="PPr")
            PP2 = PPr.rearrange("p (d t) -> p d t", d=3)
            nc.vector.tensor_mul(PP2[:, 0:2, :], DOT3[:, 3:5, 0:TC], DOT3[:, 4:6, 0:TC])
            nc.vector.tensor_mul(
                PP2[:, 2:3, :], DOT3[:, 5:6, 0:TC], DOT3[:, 3:4, 1 : TC + 1]
            )
            QQ = work.tile([P, 3 * TC], f32, tag="QQ")
            nc.vector.tensor_mul(
                QQ.rearrange("p (d t) -> p d t", d=3),
                DOT3[:, 0:3, 0:TC],
                DOT3[:, 0:3, 0:TC],
            )
            PK = work.tile([P, 6 * TC], f32, tag="PK")  # [1/(P1P2) | P1P2-Q^2]
            nc.vector.reciprocal_approx_fast(out=PK[:, 0 : 3 * TC], in_=PPr)
            nc.vector.scalar_tensor_tensor(
                PK[:, 3 * TC : 6 * TC], QQ, -1.0, PPr, op0=Alu.mult, op1=Alu.add
            )
            nc.vector.tensor_scalar_max(
                PK[:, 3 * TC : 6 * TC], PK[:, 3 * TC : 6 * TC], 0.0
            )
            SQO = work.tile([P, 6 * TC], f32, tag="SQO")  # [1/sqrt(P) | |sin|*sqrt(P)]
            nc.scalar.activation(SQO, PK, Act.Sqrt)
            SR2 = SQO[:, 0 : 3 * TC].rearrange("p (d t) -> p d t", d=3)
            CR = work.tile([P, 3 * TC], f32, tag="CR")
            nc.vector.tensor_mul(
                CR.rearrange("p (d t) -> p d t", d=3), DOT3[:, 0:3, 0:TC], SR2
            )
            CR2v = CR.rearrange("p (d t) -> p d t", d=3)
            nc.vector.tensor_scalar(
                G4[:, 3:6, :], CR2v, -1.0, 1.0, op0=Alu.max, op1=Alu.min
            )
            SMf = work.tile([P, 3 * TC], f32, tag="SMf")
            nc.vector.tensor_mul(SMf, SQO[:, 3 * TC : 6 * TC], SQO[:, 0 : 3 * TC])
            nc.vector.tensor_mul(
                G4[:, 0:3, :],
                SMf.rearrange("p (d t) -> p d t", d=3),
                SGf.rearrange("p (d t) -> p d t", d=3),
            )

            # ---- transpose on PE: GT_s[8t'+j, p] = G[p, 128s + 8t'+j] ----
            NS = 8 * TC // 128  # subtiles per chunk
            GT = work.tile([P, NS * 128], f16, tag="GT")
            GT3 = GT.rearrange("p (s r) -> p s r", s=NS)
            OUT = work.tile([P, TC * 64], f32, tag="OUT")
            for sp in range(NS // 2):
                pst = psumt.tile([P, 256], f16, tag="pst")
                for k in range(2):
                    s = 2 * sp + k
                    nc.tensor.transpose(
                        pst[:, 128 * k : 128 * (k + 1)],
                        G[:, 128 * s : 128 * (s + 1)],
                        IDsb,
                    )
                nc.vector.tensor_scalar_mul(
                    GT[:, 256 * sp : 256 * (sp + 1)], pst, 1.0
                )
            for s in range(NS):
                ps = psum.tile([P, 1024], f32, tag="ps")
                for h in range(2):
                    nc.tensor.matmul(
                        ps[:, 512 * h : 512 * (h + 1)],
                        lhsT=GT3[:, s, :],
                        rhs=W16sb[:, 512 * h : 512 * (h + 1)],
                        start=True,
                        stop=True,
                    )
                dst = OUT[:, 1024 * s : 1024 * (s + 1)]
                if ch >= 4 and s % 2 == 1:
                    nc.vector.tensor_scalar_mul(dst, ps, 1.0)
                else:
                    nc.scalar.copy(dst, ps)
                # partition p = 8*bl + r -> position base r*4096 + bl*256
                dstap = bass.AP(
                    tensor=out,
                    offset=TC * 64 * ch + 1024 * s,
                    ap=[[256 * 64, 16], [4096 * 64, 8], [1, 1024]],
                )
                nc.sync.dma_start(
                    out=dstap, in_=OUT[:, 1024 * s : 1024 * (s + 1)]
                )

    nc.compile()
    return nc


def _get_nc():
    if "nc" not in _CACHE:
        _CACHE["nc"] = _build_module()
    return _CACHE["nc"]


def _run(in_maps, trace=False, **kw):
    from concourse import bass_utils

    nc = _get_nc()
    return bass_utils.run_bass_kernel_spmd(
        nc, in_maps, core_ids=list(range(NCORES)), trace=trace, **kw
    )


def _make_in_maps(backbone_coords, W, b):
    coords = np.ascontiguousarray(backbone_coords, dtype=np.float32)
    W = np.asarray(W, dtype=np.float32)
    b = np.asarray(b, dtype=np.float32)
    # 16-block-diagonal weights: row 8t+j = feature j of block t,
    # row 8t+6 = bias (ones feature), row 8t+7 = zero.
    w16 = np.zeros((128, 1024), dtype=np.float16)
    for t in range(16):
        w16[8 * t : 8 * t + 6, 64 * t : 64 * (t + 1)] = W.T.astype(np.float16)
        w16[8 * t + 6, 64 * t : 64 * (t + 1)] = b.astype(np.float16)

    # Host-precomputed difference vectors (f16), with the reference's
    # boundary duplications baked in:
    #   d1(t) = N(t) - C(t-1)            (d1(0) = N(0) - C(0): phi dup)
    #   d2(t) = CA(t) - N(t);  d3(t) = C(t) - CA(t)
    #   d4(t) = d1(t+1); d4(L-1) = N - C (psi dup)
    #   d5(t) = d2(t+1); d5(L-1) = -d3   (omega degenerates to sin=0,cos=1)
    # plus one halo column per chunk: dk(L) continues the same rules.
    pos_pre = [sum(TCS[:c]) for c in range(NCH + 1)]
    nb = 16  # blocks per row
    in_maps = []
    for i in range(NCORES):
        arr = coords[PB * i : PB * (i + 1)].reshape(PB, L, 4, 3)
        N, CA, C = arr[:, :, 0], arr[:, :, 1], arr[:, :, 2]
        d = np.empty((5, PB, L + 1, 3), dtype=np.float32)
        d[0, :, 1:L] = N[:, 1:] - C[:, :-1]
        d[0, :, 0] = N[:, 0] - C[:, 0]
        d[1, :, :L] = CA - N
        d[2, :, :L] = C - CA
        d[3, :, : L - 1] = d[0, :, 1:L]
        d[3, :, L - 1] = N[:, -1] - C[:, -1]
        d[4, :, : L - 1] = d[1, :, 1:L]
        d[4, :, L - 1] = -d[2, :, L - 1]
        # halo col L: d1(L) = d4(L-1), d2(L) = d5(L-1); rest dummies
        d[0, :, L] = d[3, :, L - 1]
        d[1, :, L] = d[4, :, L - 1]
        d[2, :, L] = d[2, :, L - 1]
        d[3, :, L] = d[3, :, L - 1]
        d[4, :, L] = d[4, :, L - 1]
        d16 = d.astype(np.float16)
        # assemble per-chunk blocks: partition p = 8*bl + r covers
        # positions bl*256 + [POS, POS+H) of row r.
        DW = 15 * (L // nb + NCH)
        sl = np.empty((nb, PB, DW), dtype=np.float16)
        doff = 0
        for c in range(NCH):
            TC_, H_ = TCS[c], TCS[c] + 1
            base = pos_pre[c]
            for bl in range(nb):
                lo = bl * 256 + base
                blk = d16[:, :, lo : lo + H_]          # [5, PB, H, 3]
                blk = blk.transpose(1, 0, 3, 2)        # [PB, 5, 3, H]
                sl[bl, :, doff : doff + 15 * H_] = blk.reshape(PB, 15 * H_)
            doff += 15 * H_
        in_maps.append({
            "dvec": sl.reshape(P, DW),
            "w16": w16,
            "ident": np.eye(128, dtype=np.float16),
        })
    return in_maps


def kernel(backbone_coords, W, b):
    in_maps = _make_in_maps(backbone_coords, W, b)
    res = _run(in_maps)
    outs = [r["out"].reshape(PB, L, 64) for r in res.results]
    return np.concatenate(outs, axis=0)


# revision 65
# speedup vs baseline: 4.3086x; 1.0185x over previous
"""Trainium2 Bass kernel for DihedralAngleEncoder.

Computes phi/psi/omega backbone dihedral sin/cos features and projects
them 6->64 with a linear layer, for coords [64, 4096, 4, 3].

Math notes (vs. the jax reference):
  - cos(sign*arccos(c)) == c, and sin(sign*arccos(c)) == sign*sqrt(1-c^2),
    so arccos/sin/cos are never evaluated.
  - sign(n1_normalized . v3) == sign(n1 . v3) (norms are positive).
  - Shift reuse: d4(i) = d1(i+1), d5(i) = d2(i+1), c45(i) = c12(i+1),
    |c45|^2(i) = |c12|^2(i+1) -- only 3 cross products and 9 dots are
    computed, the rest are shifted column views.
  - The boundary duplications (phi at i==0, psi/omega at i==L-1) are
    realized with padded shifted loads; omega at i==L-1 degenerates to
    exactly sin=0, cos=1 which is patched in with memsets.

Sharding: pure data parallel over the batch dim, 8 batch rows per core.
Each core processes 32768 positions laid out as SBUF [128 part, 256 col]
(pos = p*256 + t), pipelined in NCH column chunks. Features are written
as fp16 into an AoS tile G[p, 8*t + j] (j: 3 sins, 3 coss, ones, zero),
batch-transposed with a single xbar DMA-transpose per chunk, and
projected on the PE against a 16-block-diagonal [128, 1024] fp16 weight.
PSUM comes out position-major; staged copies form one fully contiguous
[128, 16KB] HBM store per chunk.
"""

import sys
from contextlib import ExitStack

import numpy as np

if "/opt/trn_rl_repo" not in sys.path:
    sys.path.insert(0, "/opt/trn_rl_repo")

B, L = 64, 4096
NCORES = 8
PB = B // NCORES            # batch rows per core
NPOS = PB * L               # 32768 positions per core
P = 128                     # SBUF partitions
T = NPOS // P               # 256 cols (positions) per partition
NCH = 4                     # col chunks (pipeline stages)
TC = T // NCH               # 64 positions per partition per chunk
H = TC + 1                  # halo width for shift reuse

_CACHE = {}


def _build_module():
    import concourse.bass as bass
    import concourse.bacc as bacc
    import concourse.tile as tile
    from concourse import mybir

    f32 = mybir.dt.float32
    f16 = mybir.dt.float16
    Alu = mybir.AluOpType
    Act = mybir.ActivationFunctionType

    nc = bacc.Bacc(trn_type="TRN2")
    XW = 12 + T * 12 + 24   # left pad + main + right pad + slack
    coords = nc.dram_tensor("coords", [P, XW], f16, kind="ExternalInput")
    w16 = nc.dram_tensor("w16", [128, 1024], f16, kind="ExternalInput")
    ident = nc.dram_tensor("ident", [128, 128], f16, kind="ExternalInput")
    out = nc.dram_tensor("out", [NPOS, 64], f32, kind="ExternalOutput")

    with tile.TileContext(nc) as tc, ExitStack() as ctx:
        singles = ctx.enter_context(tc.tile_pool(name="singles", bufs=1))
        work = ctx.enter_context(tc.tile_pool(name="work", bufs=4))
        psum = ctx.enter_context(tc.tile_pool(name="psum", bufs=3, space="PSUM"))
        psumt = ctx.enter_context(tc.tile_pool(name="psumt", bufs=2, space="PSUM"))

        # ---- input: padded coords tile ----
        # cols 0..11 left pad (prev position), 12..3083 main, 3084..3095
        # right pad (next position), 3096..3103 unused.
        # Partition p covers positions (p%8)*4096 + (p//8)*256 + t, i.e.
        # batch row p%8, block p//8 (host pre-permutes coords to match and
        # precomputes the 12-col left/right neighbor pads, including the
        # boundary duplications).
        X = singles.tile([P, XW], f16)
        W16sb = singles.tile([128, 1024], f16)
        IDsb = singles.tile([128, 128], f16)
        # warm both Act function-table sets off the critical path
        TRIG = singles.tile([128, 1], f32)
        nc.scalar.memzero(TRIG)
        nc.scalar.activation(TRIG, TRIG, Act.Sqrt)
        nc.scalar.activation(TRIG, TRIG, Act.Sign)
        # coords in chunk-aligned slices so chunk c waits only on slices <= c
        bnds = [0, 800, 1568, 2336, XW]
        for c in range(NCH):
            nc.sync.dma_start(
                out=X[:, bnds[c] : bnds[c + 1]],
                in_=coords[:, bnds[c] : bnds[c + 1]],
            )
            if c == 1:
                nc.sync.dma_start(out=W16sb, in_=w16[:, :])
                nc.sync.dma_start(out=IDsb, in_=ident[:, :])

        def XV(off, ncomp, width):
            # [p, comp, t] view: comp stride 1, position stride 12
            return X[:, off : off + 12 * width].rearrange(
                "p (t c) -> p c t", c=12
            )[:, 0:ncomp, :]

        for ch in range(NCH):
            xb = 12 + 12 * TC * ch

            # ---- stage 1: difference vectors [d1,d2,d3,d4,d5] ----
            D = work.tile([P, 5 * 3 * H], f16, tag="D")
            D4 = D.rearrange("p (v k t) -> p v k t", v=5, k=3)
            # d2 = CA-N, d3 = C-CA (halo'd)
            nc.vector.tensor_sub(D4[:, 1:3, :, :], XV(xb + 3, 6, H), XV(xb, 6, H))
            # d1 = N_i - C_{i-1} (halo'd)
            nc.vector.tensor_sub(D4[:, 0:1, :, :], XV(xb, 3, H), XV(xb - 6, 3, H))
            # d4 = d1 shifted, d5 = d2 shifted (width TC)
            nc.vector.tensor_scalar_mul(
                D4[:, 3:5, :, 0:TC], D4[:, 0:2, :, 1 : TC + 1], 1.0
            )

            # ---- stage 2: cross products [c12, c23, c34] ----
            T1 = work.tile([P, 3 * 3 * H], f16, tag="T1")
            T2 = work.tile([P, 3 * 3 * H], f16, tag="T2")
            T14 = T1.rearrange("p (x k t) -> p x k t", x=3, k=3)
            T24 = T2.rearrange("p (x k t) -> p x k t", x=3, k=3)
            for k in range(3):
                p1, p2 = (k + 1) % 3, (k + 2) % 3
                nc.vector.tensor_mul(
                    T14[:, :, k, :], D4[:, 0:3, p1, :], D4[:, 1:4, p2, :]
                )
                nc.vector.tensor_mul(
                    T24[:, :, k, :], D4[:, 0:3, p2, :], D4[:, 1:4, p1, :]
                )
            XP = T1
            XP4 = T14
            nc.vector.tensor_sub(XP, T1, T2)

            # ---- stage 3: 9 dots: [A.B, B.C, C.M | n12, n23, n34 | S1,S2,S3]
            PR = work.tile([P, 9 * 3 * H], f16, tag="PR")
            PR4 = PR.rearrange("p (d k t) -> p d k t", d=9, k=3)
            nc.vector.tensor_mul(PR4[:, 0:2, :, :], XP4[:, 0:2, :, :], XP4[:, 1:3, :, :])
            # c12 shifted by one position: flat view at offset 1, k-stride H
            XPs = XP[:, 1 : 1 + 3 * H].rearrange("p (k t) -> p k t", k=3)
            nc.vector.tensor_mul(PR4[:, 2, :, :], XP4[:, 2, :, :], XPs)
            nc.vector.tensor_mul(PR4[:, 3:6, :, :], XP4[:, 0:3, :, :], XP4[:, 0:3, :, :])
            m4eng = nc.vector if (ch == 0 or ch >= 4) else nc.gpsimd
            m4eng.tensor_mul(PR4[:, 6:9, :, :], XP4[:, 0:3, :, :], D4[:, 2:5, :, :])
            DOT = work.tile([P, 9 * H], f16, tag="DOT")
            DOT3 = DOT.rearrange("p (d t) -> p d t", d=9)
            nc.vector.tensor_add(DOT3, PR4[:, :, 0, :], PR4[:, :, 1, :])
            nc.vector.tensor_add(DOT3, DOT3, PR4[:, :, 2, :])

            # ---- stage 4: angles -> sin/cos features (width TC) ----
            # cos = Q/sqrt(P1*P2); |sin| = sqrt(P1*P2 - Q^2)/sqrt(P1*P2)
            # (so both sqrts pack into ONE Act instruction); the sign of sin
            # is OR'd in from the S dots' sign bits on DVE.
            G = work.tile([P, 8 * TC], f16, tag="G")
            G4 = G.rearrange("p (t j) -> p j t", j=8)
            nc.gpsimd.memset(G4[:, 6, :], 1.0)
            nc.gpsimd.memset(G4[:, 7, :], 0.0)

            SGf = work.tile([P, 3 * TC], f32, tag="SGf")
            nc.scalar.activation(
                SGf.rearrange("p (d t) -> p d t", d=3),
                DOT3[:, 6:9, 0:TC],
                Act.Sign,
            )
            PPr = work.tile([P, 3 * TC], f32, tag="PPr")
            PP2 = PPr.rearrange("p (d t) -> p d t", d=3)
            nc.vector.tensor_mul(PP2[:, 0:2, :], DOT3[:, 3:5, 0:TC], DOT3[:, 4:6, 0:TC])
            nc.vector.tensor_mul(
                PP2[:, 2:3, :], DOT3[:, 5:6, 0:TC], DOT3[:, 3:4, 1 : TC + 1]
            )
            QQ = work.tile([P, 3 * TC], f32, tag="QQ")
            nc.vector.tensor_mul(
                QQ.rearrange("p (d t) -> p d t", d=3),
                DOT3[:, 0:3, 0:TC],
                DOT3[:, 0:3, 0:TC],
            )
            PK = work.tile([P, 6 * TC], f32, tag="PK")  # [1/(P1P2) | P1P2-Q^2]
            nc.vector.reciprocal_approx_fast(out=PK[:, 0 : 3 * TC], in_=PPr)
            nc.vector.scalar_tensor_tensor(
                PK[:, 3 * TC : 6 * TC], QQ, -1.0, PPr, op0=Alu.mult, op1=Alu.add
            )
            nc.vector.tensor_scalar_max(
                PK[:, 3 * TC : 6 * TC], PK[:, 3 * TC : 6 * TC], 0.0
            )
            SQO = work.tile([P, 6 * TC], f32, tag="SQO")  # [1/sqrt(P) | |sin|*sqrt(P)]
            nc.scalar.activation(SQO, PK, Act.Sqrt)
            SR2 = SQO[:, 0 : 3 * TC].rearrange("p (d t) -> p d t", d=3)
            CR = work.tile([P, 3 * TC], f32, tag="CR")
            nc.vector.tensor_mul(
                CR.rearrange("p (d t) -> p d t", d=3), DOT3[:, 0:3, 0:TC], SR2
            )
            CR2v = CR.rearrange("p (d t) -> p d t", d=3)
            nc.vector.tensor_scalar(
                G4[:, 3:6, :], CR2v, -1.0, 1.0, op0=Alu.max, op1=Alu.min
            )
            SMf = work.tile([P, 3 * TC], f32, tag="SMf")
            nc.vector.tensor_mul(SMf, SQO[:, 3 * TC : 6 * TC], SQO[:, 0 : 3 * TC])
            nc.vector.tensor_mul(
                G4[:, 0:3, :],
                SMf.rearrange("p (d t) -> p d t", d=3),
                SGf.rearrange("p (d t) -> p d t", d=3),
            )

            # ---- transpose on PE: GT_s[8t'+j, p] = G[p, 128s + 8t'+j] ----
            NS = 8 * TC // 128  # subtiles per chunk
            GT = work.tile([P, NS * 128], f16, tag="GT")
            GT3 = GT.rearrange("p (s r) -> p s r", s=NS)
            OUT = work.tile([P, TC * 64], f32, tag="OUT")
            for sp in range(NS // 2):
                pst = psumt.tile([P, 256], f16, tag="pst")
                for k in range(2):
                    s = 2 * sp + k
                    nc.tensor.transpose(
                        pst[:, 128 * k : 128 * (k + 1)],
                        G[:, 128 * s : 128 * (s + 1)],
                        IDsb,
                    )
                nc.vector.tensor_scalar_mul(
                    GT[:, 256 * sp : 256 * (sp + 1)], pst, 1.0
                )
            for s in range(NS):
                ps = psum.tile([P, 1024], f32, tag="ps")
                for h in range(2):
                    nc.tensor.matmul(
                        ps[:, 512 * h : 512 * (h + 1)],
                        lhsT=GT3[:, s, :],
                        rhs=W16sb[:, 512 * h : 512 * (h + 1)],
                        start=True,
                        stop=True,
                    )
                dst = OUT[:, 1024 * s : 1024 * (s + 1)]
                if ch >= 4 and s % 2 == 1:
                    nc.vector.tensor_scalar_mul(dst, ps, 1.0)
                else:
                    nc.scalar.copy(dst, ps)
                # partition p = 8*bl + r -> position base r*4096 + bl*256
                dstap = bass.AP(
                    tensor=out,
                    offset=TC * 64 * ch + 1024 * s,
                    ap=[[256 * 64, 16], [4096 * 64, 8], [1, 1024]],
                )
                nc.sync.dma_start(
                    out=dstap, in_=OUT[:, 1024 * s : 1024 * (s + 1)]
                )

    nc.compile()
    return nc


def _get_nc():
    if "nc" not in _CACHE:
        _CACHE["nc"] = _build_module()
    return _CACHE["nc"]


def _run(in_maps, trace=False, **kw):
    from concourse import bass_utils

    nc = _get_nc()
    return bass_utils.run_bass_kernel_spmd(
        nc, in_maps, core_ids=list(range(NCORES)), trace=trace, **kw
    )


def _make_in_maps(backbone_coords, W, b):
    coords = np.ascontiguousarray(backbone_coords, dtype=np.float32)
    W = np.asarray(W, dtype=np.float32)
    b = np.asarray(b, dtype=np.float32)
    # 16-block-diagonal weights: row 8t+j = feature j of block t,
    # row 8t+6 = bias (ones feature), row 8t+7 = zero.
    w16 = np.zeros((128, 1024), dtype=np.float16)
    for t in range(16):
        w16[8 * t : 8 * t + 6, 64 * t : 64 * (t + 1)] = W.T.astype(np.float16)
        w16[8 * t + 6, 64 * t : 64 * (t + 1)] = b.astype(np.float16)

    # Host-precomputed difference vectors (f16), with the reference's
    # boundary duplications baked in:
    #   d1(t) = N(t) - C(t-1)            (d1(0) = N(0) - C(0): phi dup)
    #   d2(t) = CA(t) - N(t);  d3(t) = C(t) - CA(t)
    #   d4(t) = d1(t+1); d4(L-1) = N - C (psi dup)
    #   d5(t) = d2(t+1); d5(L-1) = -d3   (omega degenerates to sin=0,cos=1)
    # plus one halo column per chunk: dk(L) continues the same rules.
    pos_pre = [sum(TCS[:c]) for c in range(NCH + 1)]
    nb = 16  # blocks per row
    in_maps = []
    for i in range(NCORES):
        arr = coords[PB * i : PB * (i + 1)].reshape(PB, L, 4, 3)
        N, CA, C = arr[:, :, 0], arr[:, :, 1], arr[:, :, 2]
        d = np.empty((5, PB, L + 1, 3), dtype=np.float32)
        d[0, :, 1:L] = N[:, 1:] - C[:, :-1]
        d[0, :, 0] = N[:, 0] - C[:, 0]
        d[1, :, :L] = CA - N
        d[2, :, :L] = C - CA
        d[3, :, : L - 1] = d[0, :, 1:L]
        d[3, :, L - 1] = N[:, -1] - C[:, -1]
        d[4, :, : L - 1] = d[1, :, 1:L]
        d[4, :, L - 1] = -d[2, :, L - 1]
        # halo col L: d1(L) = d4(L-1), d2(L) = d5(L-1); rest dummies
        d[0, :, L] = d[3, :, L - 1]
        d[1, :, L] = d[4, :, L - 1]
        d[2, :, L] = d[2, :, L - 1]
        d[3, :, L] = d[3, :, L - 1]
        d[4, :, L] = d[4, :, L - 1]
        d16 = d.astype(np.float16)
        # assemble per-chunk blocks: partition p = 8*bl + r covers
        # positions bl*256 + [POS, POS+H) of row r.
        DW = 15 * (L // nb + NCH)
        sl = np.empty((nb, PB, DW), dtype=np.float16)
        doff = 0
        for c in range(NCH):
            TC_, H_ = TCS[c], TCS[c] + 1
            base = pos_pre[c]
            for bl in range(nb):
                lo = bl * 256 + base
                blk = d16[:, :, lo : lo + H_]          # [5, PB, H, 3]
                blk = blk.transpose(1, 0, 3, 2)        # [PB, 5, 3, H]
                sl[bl, :, doff : doff + 15 * H_] = blk.reshape(PB, 15 * H_)
            doff += 15 * H_
        in_maps.append({
            "dvec": sl.reshape(P, DW),
            "w16": w16,
            "ident": np.eye(128, dtype=np.float16),
        })
    return in_maps


def kernel(backbone_coords, W, b):
    in_maps = _make_in_maps(backbone_coords, W, b)
    res = _run(in_maps)
    outs = [r["out"].reshape(PB, L, 64) for r in res.results]
    return np.concatenate(outs, axis=0)
